# revision 1
# baseline (speedup 1.0000x reference)
"""Trainium2 kernel for nn_DecoderLayer_16097537426031 (gnn_message_passing).

Math (reference):
  A1 = rownorm(adp + I), A2 = rownorm(adp.T + I)
  mixprop(x, A, W, b) = W0 h0 + W1 h1 + W2 h2 + b,  h0 = x,
    h1 = a x + B A x, h2 = a x + B A h1   (a=0.05, B=0.95)
  out_pre = mixprop(x,A1,W1,b1) + mixprop(x,A2,W2,b2) + x
  out = LayerNorm_{C,N,T}(out_pre) * ln_w[:, idx, :] + ln_b[:, idx, :]

Channel mixing (64x64) commutes with node contraction (A @ .), so with
  U0 = W10 + a(W11+W12), U1 = B(W11 + a W12), U2 = B^2 W12   (same V for W2)
  M0 = U0 + V0 + I
  out_pre = M0 x + A1 (U1 x + A1 (U2 x)) + A2 (V1 x + A2 (V2 x)) + b1 + b2
The small channel matmuls (2.6% of FLOPs) are folded on host; the four big
[3000x3000] @ [3000x1536] node-propagation matmuls per core (2 samples packed
along the free dim), the adds, and the LayerNorm run on device. Data-parallel
over batch: core c gets samples (2c, 2c+1).
"""

import numpy as np
import ml_dtypes

import concourse.bass as bass
import concourse.bacc as bacc
import concourse.mybir as mybir
from concourse.tile import TileContext
from concourse.bass_utils import run_bass_kernel_spmd

B, C, N, T = 16, 64, 3000, 12
GDEP = 2
ALPHA = 0.05
EPS = 1e-5
FT = C * T            # 768 per-sample free width
F2 = 2 * FT           # 1536: two samples packed
NE = float(C * N * T) # LayerNorm element count per sample
BF16 = ml_dtypes.bfloat16

KT = [(k * 128, min(128, N - k * 128)) for k in range((N + 127) // 128)]   # 24 k-tiles
VS = KT                                                                     # 24 v-subtiles
VP = [(v * 256, min(256, N - v * 256)) for v in range((N + 255) // 256)]    # 12 v-pairs


def _build_nc():
    nc = bacc.Bacc(num_swdge_queues=4)
    dt = mybir.dt
    f32, bf16 = dt.float32, dt.bfloat16

    a1t = nc.dram_tensor("a1t", [N, N], bf16, kind="ExternalInput")
    a2t = nc.dram_tensor("a2t", [N, N], bf16, kind="ExternalInput")
    p_in = nc.dram_tensor("p", [N, F2], bf16, kind="ExternalInput")
    q_in = nc.dram_tensor("q", [N, F2], bf16, kind="ExternalInput")
    r_in = nc.dram_tensor("r", [N, F2], bf16, kind="ExternalInput")
    s_in = nc.dram_tensor("s", [N, F2], bf16, kind="ExternalInput")
    m0x_in = nc.dram_tensor("m0x", [N, F2], f32, kind="ExternalInput")
    lnw_in = nc.dram_tensor("lnw", [N, FT], f32, kind="ExternalInput")
    lnb_in = nc.dram_tensor("lnb", [N, FT], f32, kind="ExternalInput")
    out_d = nc.dram_tensor("out", [N, F2], f32, kind="ExternalOutput")

    sub_op, mul_op = mybir.AluOpType.subtract, mybir.AluOpType.mult

    with TileContext(nc) as tc:
        with (
            tc.tile_pool(name="rhs", bufs=32) as rhs_pool,
            tc.tile_pool(name="at", bufs=8) as a_pool,
            tc.tile_pool(name="bstr", bufs=4) as bstr_pool,
            tc.tile_pool(name="fstr", bufs=2) as fstr_pool,
            tc.tile_pool(name="pre", bufs=2) as pre_pool,
            tc.tile_pool(name="of", bufs=2) as of_pool,
            tc.tile_pool(name="ln", bufs=2) as ln_pool,
            tc.tile_pool(name="scrap", bufs=2) as scrap_pool,
            tc.tile_pool(name="red", bufs=4) as red_pool,
            tc.tile_pool(name="misc", bufs=2) as misc_pool,
            tc.tile_pool(name="psmm", bufs=6, space="PSUM") as ps_pool,
            tc.tile_pool(name="psst", bufs=1, space="PSUM") as ps_stat_pool,
            tc.tile_pool(name="psbc", bufs=1, space="PSUM") as ps_bc_pool,
            tc.tile_pool(name="dacc", bufs=24, space="DRAM") as dacc_pool,
            tc.tile_pool(name="dz1", bufs=24, space="DRAM") as dz1_pool,
            tc.tile_pool(name="dz3", bufs=24, space="DRAM") as dz3_pool,
            tc.tile_pool(name="dpre", bufs=24, space="DRAM") as dpre_pool,
        ):
            ones_col = misc_pool.tile([128, 1], f32, tag="ones_col")
            nc.vector.memset(ones_col, 1.0)
            ones_row = misc_pool.tile([1, 128], f32, tag="ones_row")
            nc.vector.memset(ones_row, 1.0)

            def load_rhs(src):
                tiles = []
                for ki, (k0, kw) in enumerate(KT):
                    t = rhs_pool.tile([128, F2], bf16, tag="rhs", name=f"rhs_{k0}")
                    if isinstance(src, list):
                        nc.gpsimd.dma_start(out=t[:kw], in_=src[ki][:kw])
                    else:
                        nc.gpsimd.dma_start(out=t[:kw], in_=src[k0 : k0 + kw, :])
                    tiles.append(t)
                return tiles

            def mm_phase(aT, rhs_tiles, consume):
                """psum[v] = aT.T-chain @ rhs; consume(vs_idx, v0, vw, [ps_f0, ps_f1, ps_f2])."""
                for vp_idx, (v0, vw) in enumerate(VP):
                    nsub = 2 if vw > 128 else 1
                    subw = [128, vw - 128] if nsub == 2 else [vw]
                    ps = [
                        [
                            ps_pool.tile(
                                [128, 512], mybir.dt.float32, tag="ps",
                                name=f"ps_{vp_idx}_{_s}_{_f}",
                            )
                            for _f in range(3)
                        ]
                        for _s in range(nsub)
                    ]
                    for ki, (k0, kw) in enumerate(KT):
                        at = a_pool.tile([128, 256], mybir.dt.bfloat16, tag="at")
                        nc.gpsimd.dma_start(
                            out=at[:kw, :vw], in_=aT[k0 : k0 + kw, v0 : v0 + vw]
                        )
                        first, last = ki == 0, ki == len(KT) - 1
                        for s in range(nsub):
                            lhsT = at[:kw, 128 * s : 128 * s + subw[s]]
                            for f in range(3):
                                nc.tensor.matmul(
                                    out=ps[s][f][: subw[s]],
                                    lhsT=lhsT,
                                    rhs=rhs_tiles[ki][:kw, 512 * f : 512 * (f + 1)],
                                    start=first,
                                    stop=last,
                                )
                    for s in range(nsub):
                        consume(2 * vp_idx + s, v0 + 128 * s, subw[s], ps[s])

            # ---- Phase 1: z1q = Q + A1 @ P -> DRAM ----
            p_tiles = load_rhs(p_in)
            z1q_dram = [None] * len(VS)

            def ph1_consume(vi, v0, vw, ps):
                qt = bstr_pool.tile([128, F2], mybir.dt.bfloat16, tag="bstr")
                nc.gpsimd.dma_start(out=qt[:vw], in_=q_in[v0 : v0 + vw, :])
                z = bstr_pool.tile([128, F2], mybir.dt.bfloat16, tag="zout")
                for f in range(3):
                    sl = slice(512 * f, 512 * (f + 1))
                    nc.vector.tensor_add(z[:vw, sl], ps[f][:vw], qt[:vw, sl])
                d = dz1_pool.tile([128, F2], mybir.dt.bfloat16, tag="dz1")
                nc.gpsimd.dma_start(out=d[:vw], in_=z[:vw])
                z1q_dram[vi] = d

            mm_phase(a1t, p_tiles, ph1_consume)

            # ---- Phase 2: acc = M0X + A1 @ z1q  -> DRAM (bf16) ----
            acc_dram = [None] * len(VS)

            def ph2_consume(vi, v0, vw, ps):
                mt = fstr_pool.tile([128, F2], mybir.dt.float32, tag="fstr")
                nc.gpsimd.dma_start(out=mt[:vw], in_=m0x_in[v0 : v0 + vw, :])
                ac = bstr_pool.tile([128, F2], mybir.dt.bfloat16, tag="bstr")
                for f in range(3):
                    sl = slice(512 * f, 512 * (f + 1))
                    nc.vector.tensor_add(ac[:vw, sl], ps[f][:vw], mt[:vw, sl])
                d = dacc_pool.tile([128, F2], mybir.dt.bfloat16, tag="dacc")
                nc.gpsimd.dma_start(out=d[:vw], in_=ac[:vw])
                acc_dram[vi] = d

            z1q_tiles = load_rhs(z1q_dram)
            mm_phase(a1t, z1q_tiles, ph2_consume)

            # ---- Phase 3: z3s = S + A2 @ R ----
            r_tiles = load_rhs(r_in)
            z3s_dram = [None] * len(VS)

            def ph3_consume(vi, v0, vw, ps):
                st = bstr_pool.tile([128, F2], mybir.dt.bfloat16, tag="bstr")
                nc.gpsimd.dma_start(out=st[:vw], in_=s_in[v0 : v0 + vw, :])
                z = bstr_pool.tile([128, F2], mybir.dt.bfloat16, tag="zout")
                for f in range(3):
                    sl = slice(512 * f, 512 * (f + 1))
                    nc.vector.tensor_add(z[:vw, sl], ps[f][:vw], st[:vw, sl])
                d = dz3_pool.tile([128, F2], mybir.dt.bfloat16, tag="dz3")
                nc.gpsimd.dma_start(out=d[:vw], in_=z[:vw])
                z3s_dram[vi] = d

            mm_phase(a2t, r_tiles, ph3_consume)

            # ---- Phase 4: out_pre = acc + A2 @ z3s -> DRAM, + LN stats ----
            pre_dram = [None] * len(VS)
            stat_ps = ps_stat_pool.tile([1, 4], mybir.dt.float32, tag="psst")

            def ph4_consume(vi, v0, vw, ps):
                ac = bstr_pool.tile([128, F2], mybir.dt.bfloat16, tag="bstr")
                nc.gpsimd.dma_start(out=ac[:vw], in_=acc_dram[vi][:vw])
                pre = pre_pool.tile([128, F2], mybir.dt.float32, tag="pre")
                for f in range(3):
                    sl = slice(512 * f, 512 * (f + 1))
                    nc.vector.tensor_add(pre[:vw, sl], ps[f][:vw], ac[:vw, sl])
                d = dpre_pool.tile([128, F2], mybir.dt.float32, tag="dpre")
                nc.gpsimd.dma_start(out=d[:vw], in_=pre[:vw])
                pre_dram[vi] = d
                # stats: red cols = [sum0, sum1, sumsq0, sumsq1]
                red = red_pool.tile([128, 4], mybir.dt.float32, tag="red")
                nc.vector.reduce_sum(red[:vw, 0:1], pre[:vw, 0:FT], axis=mybir.AxisListType.X)
                nc.vector.reduce_sum(red[:vw, 1:2], pre[:vw, FT:F2], axis=mybir.AxisListType.X)
                sc0 = scrap_pool.tile([128, FT], mybir.dt.float32, tag="scrap")
                nc.scalar.activation(sc0[:vw], pre[:vw, 0:FT], mybir.ActivationFunctionType.Square, accum_out=red[:vw, 2:3])
                sc1 = scrap_pool.tile([128, FT], mybir.dt.float32, tag="scrap")
                nc.scalar.activation(sc1[:vw], pre[:vw, FT:F2], mybir.ActivationFunctionType.Square, accum_out=red[:vw, 3:4])
                nc.tensor.matmul(
                    out=stat_ps[0:1, 0:4],
                    lhsT=ones_col[:vw, 0:1],
                    rhs=red[:vw, 0:4],
                    start=(vi == 0),
                    stop=(vi == len(VS) - 1),
                )

            z3s_tiles = load_rhs(z3s_dram)
            mm_phase(a2t, z3s_tiles, ph4_consume)

            # ---- Finalize stats: mu, rinv per sample; broadcast to 128 parts ----
            f32d = mybir.dt.float32
            stat_sb = misc_pool.tile([1, 4], f32d, tag="stat_sb")
            nc.vector.tensor_copy(stat_sb, stat_ps[0:1, 0:4])
            mean2 = misc_pool.tile([1, 2], f32d, tag="mean2")
            nc.scalar.mul(mean2, stat_sb[:, 0:2], 1.0 / NE)
            ex2 = misc_pool.tile([1, 2], f32d, tag="ex2")
            nc.scalar.mul(ex2, stat_sb[:, 2:4], 1.0 / NE)
            musq = misc_pool.tile([1, 2], f32d, tag="musq")
            nc.scalar.square(musq, mean2)
            veps = misc_pool.tile([1, 2], f32d, tag="veps")
            nc.vector.tensor_sub(veps, ex2, musq)
            nc.vector.tensor_scalar_add(veps, veps, EPS)
            rec = misc_pool.tile([1, 2], f32d, tag="rec")
            nc.vector.reciprocal(rec, veps)
            fin = misc_pool.tile([1, 4], f32d, tag="fin")
            nc.scalar.copy(fin[:, 0:2], mean2)
            nc.scalar.sqrt(fin[:, 2:4], rec)
            bc_ps = ps_bc_pool.tile([128, 4], f32d, tag="psbc")
            nc.tensor.matmul(
                out=bc_ps, lhsT=ones_row[0:1, 0:128], rhs=fin[0:1, 0:4],
                start=True, stop=True,
            )
            bc = misc_pool.tile([128, 4], f32d, tag="bc")
            nc.vector.tensor_copy(bc, bc_ps)
            mu = [bc[:, 0:1], bc[:, 1:2]]
            rinv = [bc[:, 2:3], bc[:, 3:4]]

            # ---- Phase 5: normalize + affine -> out ----
            for vi, (v0, vw) in enumerate(VS):
                pre = pre_pool.tile([128, F2], f32d, tag="pre")
                nc.gpsimd.dma_start(out=pre[:vw], in_=pre_dram[vi][:vw])
                wt = ln_pool.tile([128, FT], f32d, tag="lnw")
                nc.gpsimd.dma_start(out=wt[:vw], in_=lnw_in[v0 : v0 + vw, :])
                bt = ln_pool.tile([128, FT], f32d, tag="lnb")
                nc.gpsimd.dma_start(out=bt[:vw], in_=lnb_in[v0 : v0 + vw, :])
                of = of_pool.tile([128, F2], f32d, tag="of")
                for h in range(2):
                    sl = slice(FT * h, FT * (h + 1))
                    nc.vector.tensor_scalar(
                        of[:vw, sl], pre[:vw, sl],
                        mu[h][:vw], rinv[h][:vw], sub_op, mul_op,
                    )
                    nc.vector.tensor_mul(of[:vw, sl], of[:vw, sl], wt[:vw])
                    nc.vector.tensor_add(of[:vw, sl], of[:vw, sl], bt[:vw])
                nc.gpsimd.dma_start(out=out_d[v0 : v0 + vw, :], in_=of[:vw])

    nc.compile()
    return nc


_NC_CACHE = None


def _get_nc():
    global _NC_CACHE
    if _NC_CACHE is None:
        _NC_CACHE = _build_nc()
    return _NC_CACHE


def _prep_inputs(x, adp, W1, b1, W2, b2, ln_w, ln_b, idx):
    x = np.asarray(x, dtype=np.float32)
    adp = np.asarray(adp, dtype=np.float32)
    eye = np.eye(N, dtype=np.float32)

    def rownorm(a):
        a = a + eye
        return a / a.sum(axis=1, keepdims=True)

    A1 = rownorm(adp)
    A2 = rownorm(adp.T)
    a1t = np.ascontiguousarray(A1.T).astype(BF16)
    a2t = np.ascontiguousarray(A2.T).astype(BF16)

    W1 = np.asarray(W1, dtype=np.float32)
    W2 = np.asarray(W2, dtype=np.float32)
    beta = 1.0 - ALPHA
    W10, W11, W12 = W1[:, :C], W1[:, C : 2 * C], W1[:, 2 * C :]
    W20, W21, W22 = W2[:, :C], W2[:, C : 2 * C], W2[:, 2 * C :]
    U0 = W10 + ALPHA * (W11 + W12)
    U1 = beta * (W11 + ALPHA * W12)
    U2 = (beta ** 2) * W12
    V0 = W20 + ALPHA * (W21 + W22)
    V1 = beta * (W21 + ALPHA * W22)
    V2 = (beta ** 2) * W22
    M0 = U0 + V0 + np.eye(C, dtype=np.float32)
    bias = np.asarray(b1, dtype=np.float32) + np.asarray(b2, dtype=np.float32)

    xc = x.reshape(B, C, N * T)

    def cmix(M):
        return np.matmul(M, xc)  # [B, C, N*T]

    def to_nf(a):  # [B,C,N*T] -> [B, N, C*T]
        return np.ascontiguousarray(
            a.reshape(B, C, N, T).transpose(0, 2, 1, 3).reshape(B, N, FT)
        )

    p = to_nf(cmix(U2)).astype(BF16)
    q = to_nf(cmix(U1)).astype(BF16)
    r = to_nf(cmix(V2)).astype(BF16)
    s = to_nf(cmix(V1)).astype(BF16)
    m0x = to_nf(cmix(M0) + bias[None, :, None])  # fp32

    idx = np.asarray(idx)
    lnw = np.ascontiguousarray(
        np.asarray(ln_w, dtype=np.float32)[:, idx, :].transpose(1, 0, 2).reshape(N, FT)
    )
    lnb = np.ascontiguousarray(
        np.asarray(ln_b, dtype=np.float32)[:, idx, :].transpose(1, 0, 2).reshape(N, FT)
    )

    in_maps = []
    for c in range(8):
        b0, b1i = 2 * c, 2 * c + 1
        in_maps.append(
            dict(
                a1t=a1t,
                a2t=a2t,
                p=np.hstack([p[b0], p[b1i]]),
                q=np.hstack([q[b0], q[b1i]]),
                r=np.hstack([r[b0], r[b1i]]),
                s=np.hstack([s[b0], s[b1i]]),
                m0x=np.hstack([m0x[b0], m0x[b1i]]),
                lnw=lnw,
                lnb=lnb,
            )
        )
    return in_maps


def _run(inputs, trace=False):
    nc = _get_nc()
    in_maps = _prep_inputs(**inputs)
    res = run_bass_kernel_spmd(nc, in_maps, list(range(8)), trace=trace)
    outs = np.empty((B, C, N, T), dtype=np.float32)
    for c in range(8):
        o = np.asarray(res.results[c]["out"], dtype=np.float32)
        for h in range(2):
            outs[2 * c + h] = (
                o[:, FT * h : FT * (h + 1)].reshape(N, C, T).transpose(1, 0, 2)
            )
    return outs, res


def kernel(**inputs):
    out, _ = _run(inputs, trace=False)
    return out



# revision 6
# speedup vs baseline: 1.9855x; 1.9855x over previous
"""Trainium2 kernel for nn_DecoderLayer_16097537426031 (gnn_message_passing).

Math (reference):
  A1 = rownorm(adp + I), A2 = rownorm(adp.T + I)
  mixprop(x, A, W, b) = W0 h0 + W1 h1 + W2 h2 + b,  h0 = x,
    h1 = a x + B A x, h2 = a x + B A h1   (a=0.05, B=0.95)
  out_pre = mixprop(x,A1,W1,b1) + mixprop(x,A2,W2,b2) + x
  out = LayerNorm_{C,N,T}(out_pre) * ln_w[:, idx, :] + ln_b[:, idx, :]

Channel mixing (64x64) commutes with node contraction (A @ .), so with
  U0 = W10 + a(W11+W12), U1 = B(W11 + a W12), U2 = B^2 W12   (same V for W2)
  M0 = U0 + V0 + I
  out_pre = M0 x + A1 (U1 x + A1 (U2 x)) + A2 (V1 x + A2 (V2 x)) + b1 + b2
The small channel matmuls (2.6% of FLOPs) are folded on host; the four big
[3000x3000] @ [3000x1536] node-propagation matmuls per core (2 samples packed
along the free dim), the adds, and the LayerNorm run on device.
Data-parallel over batch: core c gets samples (2c, 2c+1).

Device-side layout: all four propagation matmuls run in fp8 (e4m3) with
MatmulPerfMode.DoubleRow (two 128-row k-subtiles per instruction). A is scaled
by S=4096 on host so its ~3e-4 entries land in fp8 normal range; psum results
are scaled back by 1/S in the fused (psum*1/S + other) consume op. A1/A2 stay
resident in SBUF (one direction at a time); z intermediates stay in SBUF as
fp8 pair-tiles. acc and pre round-trip DRAM in bf16.
"""

import numpy as np
import ml_dtypes

import concourse.bass as bass
import concourse.bacc as bacc
import concourse.mybir as mybir
from concourse.tile import TileContext
from concourse.bass_utils import run_bass_kernel_spmd

B, C, N, T = 16, 64, 3000, 12
GDEP = 2
ALPHA = 0.05
EPS = 1e-5
FT = C * T            # 768 per-sample free width
F2 = 2 * FT           # 1536: two samples packed
NE = float(C * N * T) # LayerNorm element count per sample
NP = 3072             # padded node/contraction dim (12 pairs of 256)
NKP = 12              # k-pair count (DoubleRow: 256 contraction rows each)
SCALE = 4096.0        # fp8 scaling of A
INV_S = 1.0 / SCALE
BF16 = ml_dtypes.bfloat16
FP8 = ml_dtypes.float8_e4m3  # TRN fp8e4 (max +-240)

VS = [(v * 128, min(128, N - v * 128)) for v in range((N + 127) // 128)]  # 24 v-subtiles


def _build_nc(trivial_affine: bool):
    nc = bacc.Bacc(num_swdge_queues=4)
    dt = mybir.dt
    f32, bf16, f8 = dt.float32, dt.bfloat16, dt.float8e4
    DR = mybir.MatmulPerfMode.DoubleRow
    sub_op, mul_op, add_op = (
        mybir.AluOpType.subtract,
        mybir.AluOpType.mult,
        mybir.AluOpType.add,
    )

    a1_in = nc.dram_tensor("a1", [NKP, 128, 2, NP], f8, kind="ExternalInput")
    a2_in = nc.dram_tensor("a2", [NKP, 128, 2, NP], f8, kind="ExternalInput")
    p_in = nc.dram_tensor("p", [NKP, 128, 2, F2], f8, kind="ExternalInput")
    r_in = nc.dram_tensor("r", [NKP, 128, 2, F2], f8, kind="ExternalInput")
    q_in = nc.dram_tensor("q", [N, F2], f8, kind="ExternalInput")
    s_in = nc.dram_tensor("s", [N, F2], f8, kind="ExternalInput")
    m0x_in = nc.dram_tensor("m0x", [N, F2], bf16, kind="ExternalInput")
    if not trivial_affine:
        lnw_in = nc.dram_tensor("lnw", [N, FT], f32, kind="ExternalInput")
        lnb_in = nc.dram_tensor("lnb", [N, FT], f32, kind="ExternalInput")
    out_d = nc.dram_tensor("out", [N, F2], bf16, kind="ExternalOutput")

    from contextlib import ExitStack

    with TileContext(nc) as tc:
        with ExitStack() as stack:
            ep = lambda **kw: stack.enter_context(tc.tile_pool(**kw))
            a_pool = ep(name="apool", bufs=13)
            x_pool = ep(name="xpool", bufs=12)
            z_pool = ep(name="zpool", bufs=13)
            qs_pool = ep(name="qs", bufs=2)
            mx_pool = ep(name="mx", bufs=2)
            accw_pool = ep(name="accw", bufs=2)
            accr_pool = ep(name="accr", bufs=2)
            prew_pool = ep(name="prew", bufs=2)
            prer_pool = ep(name="prer", bufs=2)
            out_pool = ep(name="outp", bufs=2)
            ln_pool = ep(name="lnp", bufs=4)
            red_pool = ep(name="red", bufs=4)
            scrap_pool = ep(name="scrap", bufs=2)
            misc_pool = ep(name="misc", bufs=2)
            ps_pool = ep(name="psmm", bufs=6, space="PSUM")
            ps_stat_pool = ep(name="psst", bufs=1, space="PSUM")
            ps_bc_pool = ep(name="psbc", bufs=1, space="PSUM")
            dacc_pool = ep(name="dacc", bufs=24, space="DRAM")
            dpre_pool = ep(name="dpre", bufs=24, space="DRAM")
            ones_col = misc_pool.tile([128, 1], f32, tag="ones_col")
            nc.vector.memset(ones_col, 1.0)
            ones_row = misc_pool.tile([1, 128], f32, tag="ones_row")
            nc.vector.memset(ones_row, 1.0)

            def load_pairs(pool, src, width, tag):
                tiles = []
                for j in range(NKP):
                    t = pool.tile([128, 2, width], f8, tag=tag, name=f"{tag}_{j}")
                    nc.gpsimd.dma_start(out=t, in_=src[j])
                    tiles.append(t)
                return tiles

            def mm_phase(a_tiles, rhs_tiles, consume):
                """ps[f] = sum_j a_tiles[j].T-pairs @ rhs_tiles[j] f-chunks."""
                for vi, (v0, vw) in enumerate(VS):
                    ps = [
                        ps_pool.tile([128, 512], f32, tag="ps", name=f"ps_{vi}_{f}")
                        for f in range(3)
                    ]
                    for j in range(NKP):
                        lhsT = a_tiles[j][:, :, v0 : v0 + vw]
                        for f in range(3):
                            nc.tensor.matmul(
                                out=ps[f][:vw],
                                lhsT=lhsT,
                                rhs=rhs_tiles[j][:, :, 512 * f : 512 * (f + 1)],
                                start=(j == 0),
                                stop=(j == NKP - 1),
                                perf_mode=DR,
                            )
                    consume(vi, v0, vw, ps)

            # ---- Phase 1: z1 = Q + A1 @ P  (SBUF fp8 pair tiles) ----
            a1_t = load_pairs(a_pool, a1_in, NP, "a1")
            p_t = load_pairs(x_pool, p_in, F2, "xp")
            z1_t = [
                z_pool.tile([128, 2, F2], f8, tag="z", name=f"z1_{j}")
                for j in range(NKP)
            ]
            # rows 3000..3071 of the k dim must be zero when z1 is used as rhs
            # (partition base must be 32-aligned, so clear the whole last tile
            # before the consumes overwrite the valid rows)
            nc.vector.memset(z1_t[NKP - 1], 0.0)

            def ph1(vi, v0, vw, ps):
                qt = qs_pool.tile([128, F2], f8, tag="qs")
                nc.gpsimd.dma_start(out=qt[:vw], in_=q_in[v0 : v0 + vw, :])
                j, h = divmod(vi, 2)
                for f in range(3):
                    sl = slice(512 * f, 512 * (f + 1))
                    nc.vector.scalar_tensor_tensor(
                        out=z1_t[j][:vw, h, sl],
                        in0=ps[f][:vw],
                        scalar=INV_S,
                        in1=qt[:vw, sl],
                        op0=mul_op,
                        op1=add_op,
                    )

            mm_phase(a1_t, p_t, ph1)

            # ---- Phase 2: acc = M0X + A1 @ z1 -> DRAM bf16 ----
            acc_dram = [None] * len(VS)

            def ph2(vi, v0, vw, ps):
                mt = mx_pool.tile([128, F2], bf16, tag="mx")
                nc.gpsimd.dma_start(out=mt[:vw], in_=m0x_in[v0 : v0 + vw, :])
                at = accw_pool.tile([128, F2], bf16, tag="accw")
                for f in range(3):
                    sl = slice(512 * f, 512 * (f + 1))
                    nc.vector.scalar_tensor_tensor(
                        out=at[:vw, sl],
                        in0=ps[f][:vw],
                        scalar=INV_S,
                        in1=mt[:vw, sl],
                        op0=mul_op,
                        op1=add_op,
                    )
                d = dacc_pool.tile([128, F2], bf16, tag="dacc")
                nc.gpsimd.dma_start(out=d[:vw], in_=at[:vw])
                acc_dram[vi] = d

            mm_phase(a1_t, z1_t, ph2)

            # ---- Phase 3: z3 = S + A2 @ R ----
            a2_t = load_pairs(a_pool, a2_in, NP, "a1")
            r_t = load_pairs(x_pool, r_in, F2, "xp")
            z3_t = [
                z_pool.tile([128, 2, F2], f8, tag="z", name=f"z3_{j}")
                for j in range(NKP)
            ]
            nc.vector.memset(z3_t[NKP - 1], 0.0)

            def ph3(vi, v0, vw, ps):
                st = qs_pool.tile([128, F2], f8, tag="qs")
                nc.gpsimd.dma_start(out=st[:vw], in_=s_in[v0 : v0 + vw, :])
                j, h = divmod(vi, 2)
                for f in range(3):
                    sl = slice(512 * f, 512 * (f + 1))
                    nc.vector.scalar_tensor_tensor(
                        out=z3_t[j][:vw, h, sl],
                        in0=ps[f][:vw],
                        scalar=INV_S,
                        in1=st[:vw, sl],
                        op0=mul_op,
                        op1=add_op,
                    )

            mm_phase(a2_t, r_t, ph3)

            # ---- Phase 4: pre = acc + A2 @ z3 -> DRAM bf16, + LN stats ----
            pre_dram = [None] * len(VS)
            stat_ps = ps_stat_pool.tile([1, 6], f32, tag="psst")
            # stt chunks: (ps idx, ps col slice, out col slice, red col)
            CH = [
                (0, slice(0, 512), slice(0, 512), 0),
                (1, slice(0, 256), slice(512, 768), 1),
                (1, slice(256, 512), slice(768, 1024), 2),
                (2, slice(0, 512), slice(1024, 1536), 3),
            ]

            def ph4(vi, v0, vw, ps):
                ar = accr_pool.tile([128, F2], bf16, tag="accr")
                nc.gpsimd.dma_start(out=ar[:vw], in_=acc_dram[vi][:vw])
                pt = prew_pool.tile([128, F2], bf16, tag="prew")
                red = red_pool.tile([128, 6], f32, tag="red")
                for pi, psl, osl, rc in CH:
                    nc.vector.scalar_tensor_tensor(
                        out=pt[:vw, osl],
                        in0=ps[pi][:vw, psl],
                        scalar=INV_S,
                        in1=ar[:vw, osl],
                        op0=mul_op,
                        op1=add_op,
                        accum_out=red[:vw, rc : rc + 1],
                    )
                sc0 = scrap_pool.tile([128, FT], bf16, tag="scrap")
                nc.scalar.activation(
                    sc0[:vw], pt[:vw, 0:FT],
                    mybir.ActivationFunctionType.Square,
                    accum_out=red[:vw, 4:5],
                )
                sc1 = scrap_pool.tile([128, FT], bf16, tag="scrap")
                nc.scalar.activation(
                    sc1[:vw], pt[:vw, FT:F2],
                    mybir.ActivationFunctionType.Square,
                    accum_out=red[:vw, 5:6],
                )
                nc.tensor.matmul(
                    out=stat_ps[0:1, 0:6],
                    lhsT=ones_col[:vw, 0:1],
                    rhs=red[:vw, 0:6],
                    start=(vi == 0),
                    stop=(vi == len(VS) - 1),
                )
                d = dpre_pool.tile([128, F2], bf16, tag="dpre")
                nc.gpsimd.dma_start(out=d[:vw], in_=pt[:vw])
                pre_dram[vi] = d

            mm_phase(a2_t, z3_t, ph4)

            # ---- Finalize stats: mu, rinv per sample; broadcast to 128 ----
            stat_sb = misc_pool.tile([1, 6], f32, tag="stat_sb")
            nc.vector.tensor_copy(stat_sb, stat_ps[0:1, 0:6])
            sums2 = misc_pool.tile([1, 2], f32, tag="sums2")
            nc.vector.tensor_add(sums2[:, 0:1], stat_sb[:, 0:1], stat_sb[:, 1:2])
            nc.vector.tensor_add(sums2[:, 1:2], stat_sb[:, 2:3], stat_sb[:, 3:4])
            mean2 = misc_pool.tile([1, 2], f32, tag="mean2")
            nc.scalar.mul(mean2, sums2, 1.0 / NE)
            ex2 = misc_pool.tile([1, 2], f32, tag="ex2")
            nc.scalar.mul(ex2, stat_sb[:, 4:6], 1.0 / NE)
            musq = misc_pool.tile([1, 2], f32, tag="musq")
            nc.scalar.square(musq, mean2)
            veps = misc_pool.tile([1, 2], f32, tag="veps")
            nc.vector.tensor_sub(veps, ex2, musq)
            nc.vector.tensor_scalar_add(veps, veps, EPS)
            rec = misc_pool.tile([1, 2], f32, tag="rec")
            nc.vector.reciprocal(rec, veps)
            fin = misc_pool.tile([1, 4], f32, tag="fin")
            nc.scalar.copy(fin[:, 0:2], mean2)
            nc.scalar.sqrt(fin[:, 2:4], rec)
            bc_ps = ps_bc_pool.tile([128, 4], f32, tag="psbc")
            nc.tensor.matmul(
                out=bc_ps, lhsT=ones_row[0:1, 0:128], rhs=fin[0:1, 0:4],
                start=True, stop=True,
            )
            bc = misc_pool.tile([128, 4], f32, tag="bc")
            nc.vector.tensor_copy(bc, bc_ps)
            mu = [bc[:, 0:1], bc[:, 1:2]]
            rinv = [bc[:, 2:3], bc[:, 3:4]]

            # ---- Phase 5: normalize (+ affine) -> out ----
            for vi, (v0, vw) in enumerate(VS):
                pr = prer_pool.tile([128, F2], bf16, tag="prer")
                nc.gpsimd.dma_start(out=pr[:vw], in_=pre_dram[vi][:vw])
                of = out_pool.tile([128, F2], bf16, tag="outp")
                if trivial_affine:
                    for h in range(2):
                        sl = slice(FT * h, FT * (h + 1))
                        nc.vector.tensor_scalar(
                            of[:vw, sl], pr[:vw, sl],
                            mu[h][:vw], rinv[h][:vw], sub_op, mul_op,
                        )
                else:
                    wt = ln_pool.tile([128, FT], f32, tag="lnw")
                    nc.gpsimd.dma_start(out=wt[:vw], in_=lnw_in[v0 : v0 + vw, :])
                    bt = ln_pool.tile([128, FT], f32, tag="lnb")
                    nc.gpsimd.dma_start(out=bt[:vw], in_=lnb_in[v0 : v0 + vw, :])
                    for h in range(2):
                        sl = slice(FT * h, FT * (h + 1))
                        nc.vector.tensor_scalar(
                            of[:vw, sl], pr[:vw, sl],
                            mu[h][:vw], rinv[h][:vw], sub_op, mul_op,
                        )
                        nc.vector.tensor_mul(of[:vw, sl], of[:vw, sl], wt[:vw])
                        nc.vector.tensor_add(of[:vw, sl], of[:vw, sl], bt[:vw])
                nc.gpsimd.dma_start(out=out_d[v0 : v0 + vw, :], in_=of[:vw])

    nc.compile()
    return nc


_NC_CACHE = {}


def _get_nc(trivial_affine):
    if trivial_affine not in _NC_CACHE:
        _NC_CACHE[trivial_affine] = _build_nc(trivial_affine)
    return _NC_CACHE[trivial_affine]


def _pack_pairs(arr, width):
    """[rows<=NP, cols<=width] -> [NKP, 128, 2, width] fp8 (zero padded)."""
    z = np.zeros((NP, width), dtype=np.float32)
    z[: arr.shape[0], : arr.shape[1]] = arr
    z = z.reshape(NKP, 2, 128, width).transpose(0, 2, 1, 3)
    return np.ascontiguousarray(np.clip(z, -240.0, 240.0)).astype(FP8)


def _prep_inputs(x, adp, W1, b1, W2, b2, ln_w, ln_b, idx):
    x = np.asarray(x, dtype=np.float32)
    adp = np.asarray(adp, dtype=np.float32)
    eye = np.eye(N, dtype=np.float32)

    def rownorm(a):
        a = a + eye
        return a / a.sum(axis=1, keepdims=True)

    A1 = rownorm(adp)
    A2 = rownorm(adp.T)
    a1_pk = _pack_pairs(A1.T * SCALE, NP)
    a2_pk = _pack_pairs(A2.T * SCALE, NP)

    W1 = np.asarray(W1, dtype=np.float32)
    W2 = np.asarray(W2, dtype=np.float32)
    beta = 1.0 - ALPHA
    W10, W11, W12 = W1[:, :C], W1[:, C : 2 * C], W1[:, 2 * C :]
    W20, W21, W22 = W2[:, :C], W2[:, C : 2 * C], W2[:, 2 * C :]
    U0 = W10 + ALPHA * (W11 + W12)
    U1 = beta * (W11 + ALPHA * W12)
    U2 = (beta ** 2) * W12
    V0 = W20 + ALPHA * (W21 + W22)
    V1 = beta * (W21 + ALPHA * W22)
    V2 = (beta ** 2) * W22
    M0 = U0 + V0 + np.eye(C, dtype=np.float32)
    bias = np.asarray(b1, dtype=np.float32) + np.asarray(b2, dtype=np.float32)

    xc = x.reshape(B, C, N * T)

    def cmix(M):
        return np.matmul(M, xc)  # [B, C, N*T]

    def to_nf(a):  # [B,C,N*T] -> [B, N, C*T]
        return np.ascontiguousarray(
            a.reshape(B, C, N, T).transpose(0, 2, 1, 3).reshape(B, N, FT)
        )

    p = to_nf(cmix(U2))
    q = to_nf(cmix(U1))
    r = to_nf(cmix(V2))
    s = to_nf(cmix(V1))
    m0x = to_nf(cmix(M0) + bias[None, :, None])

    idx = np.asarray(idx)
    lw = np.asarray(ln_w, dtype=np.float32)[:, idx, :]
    lb = np.asarray(ln_b, dtype=np.float32)[:, idx, :]
    trivial = bool(np.all(lw == 1.0)) and bool(np.all(lb == 0.0))
    if not trivial:
        lnw = np.ascontiguousarray(lw.transpose(1, 0, 2).reshape(N, FT))
        lnb = np.ascontiguousarray(lb.transpose(1, 0, 2).reshape(N, FT))

    in_maps = []
    for c in range(8):
        b0, b1i = 2 * c, 2 * c + 1
        im = dict(
            a1=a1_pk,
            a2=a2_pk,
            p=_pack_pairs(np.hstack([p[b0], p[b1i]]), F2),
            r=_pack_pairs(np.hstack([r[b0], r[b1i]]), F2),
            q=np.hstack([q[b0], q[b1i]]).astype(FP8),
            s=np.hstack([s[b0], s[b1i]]).astype(FP8),
            m0x=np.hstack([m0x[b0], m0x[b1i]]).astype(BF16),
        )
        if not trivial:
            im["lnw"] = lnw
            im["lnb"] = lnb
        in_maps.append(im)
    return in_maps, trivial


def _run(inputs, trace=False):
    in_maps, trivial = _prep_inputs(**inputs)
    nc = _get_nc(trivial)
    res = run_bass_kernel_spmd(nc, in_maps, list(range(8)), trace=trace)
    outs = np.empty((B, C, N, T), dtype=np.float32)
    for c in range(8):
        o = np.asarray(res.results[c]["out"]).astype(np.float32)
        for h in range(2):
            outs[2 * c + h] = (
                o[:, FT * h : FT * (h + 1)].reshape(N, C, T).transpose(1, 0, 2)
            )
    return outs, res


def kernel(**inputs):
    out, _ = _run(inputs, trace=False)
    return out


# revision 7
# speedup vs baseline: 2.0422x; 1.0286x over previous
"""Trainium2 kernel for nn_DecoderLayer_16097537426031 (gnn_message_passing).

Math (reference):
  A1 = rownorm(adp + I), A2 = rownorm(adp.T + I)
  mixprop(x, A, W, b) = W0 h0 + W1 h1 + W2 h2 + b,  h0 = x,
    h1 = a x + B A x, h2 = a x + B A h1   (a=0.05, B=0.95)
  out_pre = mixprop(x,A1,W1,b1) + mixprop(x,A2,W2,b2) + x
  out = LayerNorm_{C,N,T}(out_pre) * ln_w[:, idx, :] + ln_b[:, idx, :]

Channel mixing (64x64) commutes with node contraction (A @ .), so with
  U0 = W10 + a(W11+W12), U1 = B(W11 + a W12), U2 = B^2 W12   (same V for W2)
  M0 = U0 + V0 + I
  out_pre = M0 x + A1 (U1 x + A1 (U2 x)) + A2 (V1 x + A2 (V2 x)) + b1 + b2
The small channel matmuls (2.6% of FLOPs) are folded on host; the four big
[3000x3000] @ [3000x1536] node-propagation matmuls per core (2 samples packed
along the free dim), the adds, and the LayerNorm run on device.
Data-parallel over batch: core c gets samples (2c, 2c+1).

Device-side layout: all four propagation matmuls run in fp8 (e4m3) with
MatmulPerfMode.DoubleRow (two 128-row k-subtiles per instruction). A is scaled
by S=4096 on host so its ~3e-4 entries land in fp8 normal range; psum results
are scaled back by 1/S in the fused (psum*1/S + other) consume op. A1/A2 stay
resident in SBUF (one direction at a time); z intermediates stay in SBUF as
fp8 pair-tiles. acc and pre round-trip DRAM in bf16.
"""

import numpy as np
import ml_dtypes

import concourse.bass as bass
import concourse.bacc as bacc
import concourse.mybir as mybir
from concourse.tile import TileContext
from concourse.bass_utils import run_bass_kernel_spmd

B, C, N, T = 16, 64, 3000, 12
GDEP = 2
ALPHA = 0.05
EPS = 1e-5
FT = C * T            # 768 per-sample free width
F2 = 2 * FT           # 1536: two samples packed
NE = float(C * N * T) # LayerNorm element count per sample
NP = 3072             # padded node/contraction dim (12 pairs of 256)
NKP = 12              # k-pair count (DoubleRow: 256 contraction rows each)
SCALE = 4096.0        # fp8 scaling of A
INV_S = 1.0 / SCALE
BF16 = ml_dtypes.bfloat16
FP8 = ml_dtypes.float8_e4m3  # TRN fp8e4 (max +-240)

VS = [(v * 128, min(128, N - v * 128)) for v in range((N + 127) // 128)]  # 24 v-subtiles


def _build_nc(trivial_affine: bool):
    nc = bacc.Bacc(num_swdge_queues=4)
    dt = mybir.dt
    f32, bf16, f8 = dt.float32, dt.bfloat16, dt.float8e4
    DR = mybir.MatmulPerfMode.DoubleRow
    sub_op, mul_op, add_op = (
        mybir.AluOpType.subtract,
        mybir.AluOpType.mult,
        mybir.AluOpType.add,
    )

    a1_in = nc.dram_tensor("a1", [NKP, 128, 2, NP], f8, kind="ExternalInput")
    a2_in = nc.dram_tensor("a2", [NKP, 128, 2, NP], f8, kind="ExternalInput")
    p_in = nc.dram_tensor("p", [NKP, 128, 2, F2], f8, kind="ExternalInput")
    r_in = nc.dram_tensor("r", [NKP, 128, 2, F2], f8, kind="ExternalInput")
    q_in = nc.dram_tensor("q", [N, F2], f8, kind="ExternalInput")
    s_in = nc.dram_tensor("s", [N, F2], f8, kind="ExternalInput")
    m0x_in = nc.dram_tensor("m0x", [N, F2], bf16, kind="ExternalInput")
    if not trivial_affine:
        lnw_in = nc.dram_tensor("lnw", [N, FT], f32, kind="ExternalInput")
        lnb_in = nc.dram_tensor("lnb", [N, FT], f32, kind="ExternalInput")
    out_d = nc.dram_tensor("out", [N, F2], bf16, kind="ExternalOutput")

    from contextlib import ExitStack

    with TileContext(nc) as tc:
        with ExitStack() as stack:
            ep = lambda **kw: stack.enter_context(tc.tile_pool(**kw))
            a_pool = ep(name="apool", bufs=14)
            x_pool = ep(name="xpool", bufs=12)
            z_pool = ep(name="zpool", bufs=13)
            qs_pool = ep(name="qs", bufs=2)
            mx_pool = ep(name="mx", bufs=2)
            accw_pool = ep(name="accw", bufs=2)
            accr_pool = ep(name="accr", bufs=2)
            prew_pool = ep(name="prew", bufs=2)
            prer_pool = ep(name="prer", bufs=3)
            out_pool = ep(name="outp", bufs=2)
            ln_pool = ep(name="lnp", bufs=4)
            red_pool = ep(name="red", bufs=4)
            scrap_pool = ep(name="scrap", bufs=2)
            misc_pool = ep(name="misc", bufs=2)
            ps_pool = ep(name="psmm", bufs=6, space="PSUM")
            ps_stat_pool = ep(name="psst", bufs=1, space="PSUM")
            ps_bc_pool = ep(name="psbc", bufs=1, space="PSUM")
            dacc_pool = ep(name="dacc", bufs=24, space="DRAM")
            dpre_pool = ep(name="dpre", bufs=24, space="DRAM")
            ones_col = misc_pool.tile([128, 1], f32, tag="ones_col")
            nc.vector.memset(ones_col, 1.0)
            ones_row = misc_pool.tile([1, 128], f32, tag="ones_row")
            nc.vector.memset(ones_row, 1.0)

            def load_pairs(pool, src, width, tag):
                tiles = []
                for j in range(NKP):
                    t = pool.tile([128, 2, width], f8, tag=tag, name=f"{tag}_{j}")
                    nc.gpsimd.dma_start(out=t, in_=src[j])
                    tiles.append(t)
                return tiles

            def mm_phase(a_tiles, rhs_tiles, consume):
                """ps[f] = sum_j a_tiles[j].T-pairs @ rhs_tiles[j] f-chunks."""
                for vi, (v0, vw) in enumerate(VS):
                    ps = [
                        ps_pool.tile([128, 512], f32, tag="ps", name=f"ps_{vi}_{f}")
                        for f in range(3)
                    ]
                    for j in range(NKP):
                        lhsT = a_tiles[j][:, :, v0 : v0 + vw]
                        for f in range(3):
                            nc.tensor.matmul(
                                out=ps[f][:vw],
                                lhsT=lhsT,
                                rhs=rhs_tiles[j][:, :, 512 * f : 512 * (f + 1)],
                                start=(j == 0),
                                stop=(j == NKP - 1),
                                perf_mode=DR,
                            )
                    consume(vi, v0, vw, ps)

            # ---- Phase 1: z1 = Q + A1 @ P  (SBUF fp8 pair tiles) ----
            a1_t, p_t = [], []
            for j in range(NKP):
                t = a_pool.tile([128, 2, NP], f8, tag="a1", name=f"a1_{j}")
                nc.gpsimd.dma_start(out=t, in_=a1_in[j])
                a1_t.append(t)
                t = x_pool.tile([128, 2, F2], f8, tag="xp", name=f"xp_{j}")
                nc.gpsimd.dma_start(out=t, in_=p_in[j])
                p_t.append(t)
            z1_t = [
                z_pool.tile([128, 2, F2], f8, tag="z", name=f"z1_{j}")
                for j in range(NKP)
            ]
            # rows 3000..3071 of the k dim must be zero when z1 is used as rhs
            # (partition base must be 32-aligned, so clear the whole last tile
            # before the consumes overwrite the valid rows)
            nc.vector.memset(z1_t[NKP - 1], 0.0)

            def ph1(vi, v0, vw, ps):
                qt = qs_pool.tile([128, F2], f8, tag="qs")
                nc.gpsimd.dma_start(out=qt[:vw], in_=q_in[v0 : v0 + vw, :])
                j, h = divmod(vi, 2)
                for f in range(3):
                    sl = slice(512 * f, 512 * (f + 1))
                    nc.vector.scalar_tensor_tensor(
                        out=z1_t[j][:vw, h, sl],
                        in0=ps[f][:vw],
                        scalar=INV_S,
                        in1=qt[:vw, sl],
                        op0=mul_op,
                        op1=add_op,
                    )

            mm_phase(a1_t, p_t, ph1)

            # ---- Phase 2: acc = M0X + A1 @ z1 -> DRAM bf16 ----
            acc_dram = [None] * len(VS)

            def ph2(vi, v0, vw, ps):
                mt = mx_pool.tile([128, F2], bf16, tag="mx")
                nc.gpsimd.dma_start(out=mt[:vw], in_=m0x_in[v0 : v0 + vw, :])
                at = accw_pool.tile([128, F2], bf16, tag="accw")
                for f in range(3):
                    sl = slice(512 * f, 512 * (f + 1))
                    nc.vector.scalar_tensor_tensor(
                        out=at[:vw, sl],
                        in0=ps[f][:vw],
                        scalar=INV_S,
                        in1=mt[:vw, sl],
                        op0=mul_op,
                        op1=add_op,
                    )
                d = dacc_pool.tile([128, F2], bf16, tag="dacc")
                nc.gpsimd.dma_start(out=d[:vw], in_=at[:vw])
                acc_dram[vi] = d

            mm_phase(a1_t, z1_t, ph2)

            # ---- Phase 3: z3 = S + A2 @ R ----
            a2_t = load_pairs(a_pool, a2_in, NP, "a1")
            r_t = load_pairs(x_pool, r_in, F2, "xp")
            z3_t = [
                z_pool.tile([128, 2, F2], f8, tag="z", name=f"z3_{j}")
                for j in range(NKP)
            ]
            nc.vector.memset(z3_t[NKP - 1], 0.0)

            def ph3(vi, v0, vw, ps):
                st = qs_pool.tile([128, F2], f8, tag="qs")
                nc.gpsimd.dma_start(out=st[:vw], in_=s_in[v0 : v0 + vw, :])
                j, h = divmod(vi, 2)
                for f in range(3):
                    sl = slice(512 * f, 512 * (f + 1))
                    nc.vector.scalar_tensor_tensor(
                        out=z3_t[j][:vw, h, sl],
                        in0=ps[f][:vw],
                        scalar=INV_S,
                        in1=st[:vw, sl],
                        op0=mul_op,
                        op1=add_op,
                    )

            mm_phase(a2_t, r_t, ph3)

            # ---- Phase 4: pre = acc + A2 @ z3 -> DRAM bf16, + LN stats ----
            pre_dram = [None] * len(VS)
            pre_sbuf = [None] * len(VS)
            stat_ps = ps_stat_pool.tile([1, 6], f32, tag="psst")
            # stt chunks: (ps idx, ps col slice, out col slice, red col)
            CH = [
                (0, slice(0, 512), slice(0, 512), 0),
                (1, slice(0, 256), slice(512, 768), 1),
                (1, slice(256, 512), slice(768, 1024), 2),
                (2, slice(0, 512), slice(1024, 1536), 3),
            ]

            def ph4(vi, v0, vw, ps):
                ar = accr_pool.tile([128, F2], bf16, tag="accr")
                nc.gpsimd.dma_start(out=ar[:vw], in_=acc_dram[vi][:vw])
                if vi < 12:
                    pt = prew_pool.tile([128, F2], bf16, tag="prew")
                else:
                    pt = x_pool.tile([128, F2], bf16, tag="xp", name=f"pre_sb_{vi}")
                red = red_pool.tile([128, 6], f32, tag="red")
                for pi, psl, osl, rc in CH:
                    nc.vector.scalar_tensor_tensor(
                        out=pt[:vw, osl],
                        in0=ps[pi][:vw, psl],
                        scalar=INV_S,
                        in1=ar[:vw, osl],
                        op0=mul_op,
                        op1=add_op,
                        accum_out=red[:vw, rc : rc + 1],
                    )
                sc0 = scrap_pool.tile([128, FT], bf16, tag="scrap")
                nc.scalar.activation(
                    sc0[:vw], pt[:vw, 0:FT],
                    mybir.ActivationFunctionType.Square,
                    accum_out=red[:vw, 4:5],
                )
                sc1 = scrap_pool.tile([128, FT], bf16, tag="scrap")
                nc.scalar.activation(
                    sc1[:vw], pt[:vw, FT:F2],
                    mybir.ActivationFunctionType.Square,
                    accum_out=red[:vw, 5:6],
                )
                nc.tensor.matmul(
                    out=stat_ps[0:1, 0:6],
                    lhsT=ones_col[:vw, 0:1],
                    rhs=red[:vw, 0:6],
                    start=(vi == 0),
                    stop=(vi == len(VS) - 1),
                )
                if vi < 12:
                    d = dpre_pool.tile([128, F2], bf16, tag="dpre")
                    nc.gpsimd.dma_start(out=d[:vw], in_=pt[:vw])
                    pre_dram[vi] = d
                else:
                    pre_sbuf[vi] = pt

            mm_phase(a2_t, z3_t, ph4)

            # ---- Finalize stats: mu, rinv per sample; broadcast to 128 ----
            stat_sb = misc_pool.tile([1, 6], f32, tag="stat_sb")
            nc.vector.tensor_copy(stat_sb, stat_ps[0:1, 0:6])
            sums2 = misc_pool.tile([1, 2], f32, tag="sums2")
            nc.vector.tensor_add(sums2[:, 0:1], stat_sb[:, 0:1], stat_sb[:, 1:2])
            nc.vector.tensor_add(sums2[:, 1:2], stat_sb[:, 2:3], stat_sb[:, 3:4])
            mean2 = misc_pool.tile([1, 2], f32, tag="mean2")
            nc.scalar.mul(mean2, sums2, 1.0 / NE)
            ex2 = misc_pool.tile([1, 2], f32, tag="ex2")
            nc.scalar.mul(ex2, stat_sb[:, 4:6], 1.0 / NE)
            musq = misc_pool.tile([1, 2], f32, tag="musq")
            nc.scalar.square(musq, mean2)
            veps = misc_pool.tile([1, 2], f32, tag="veps")
            nc.vector.tensor_sub(veps, ex2, musq)
            nc.vector.tensor_scalar_add(veps, veps, EPS)
            rec = misc_pool.tile([1, 2], f32, tag="rec")
            nc.vector.reciprocal(rec, veps)
            fin = misc_pool.tile([1, 4], f32, tag="fin")
            nc.scalar.copy(fin[:, 0:2], mean2)
            nc.scalar.sqrt(fin[:, 2:4], rec)
            bc_ps = ps_bc_pool.tile([128, 4], f32, tag="psbc")
            nc.tensor.matmul(
                out=bc_ps, lhsT=ones_row[0:1, 0:128], rhs=fin[0:1, 0:4],
                start=True, stop=True,
            )
            bc = misc_pool.tile([128, 4], f32, tag="bc")
            nc.vector.tensor_copy(bc, bc_ps)
            mu = [bc[:, 0:1], bc[:, 1:2]]
            rinv = [bc[:, 2:3], bc[:, 3:4]]

            # ---- Phase 5: normalize (+ affine) -> out ----
            for vi in list(range(12, len(VS))) + list(range(12)):
                v0, vw = VS[vi]
                if pre_sbuf[vi] is not None:
                    pr = pre_sbuf[vi]
                else:
                    pr = prer_pool.tile([128, F2], bf16, tag="prer")
                    nc.gpsimd.dma_start(out=pr[:vw], in_=pre_dram[vi][:vw])
                of = out_pool.tile([128, F2], bf16, tag="outp")
                if trivial_affine:
                    for h in range(2):
                        sl = slice(FT * h, FT * (h + 1))
                        nc.vector.tensor_scalar(
                            of[:vw, sl], pr[:vw, sl],
                            mu[h][:vw], rinv[h][:vw], sub_op, mul_op,
                        )
                else:
                    wt = ln_pool.tile([128, FT], f32, tag="lnw")
                    nc.gpsimd.dma_start(out=wt[:vw], in_=lnw_in[v0 : v0 + vw, :])
                    bt = ln_pool.tile([128, FT], f32, tag="lnb")
                    nc.gpsimd.dma_start(out=bt[:vw], in_=lnb_in[v0 : v0 + vw, :])
                    for h in range(2):
                        sl = slice(FT * h, FT * (h + 1))
                        nc.vector.tensor_scalar(
                            of[:vw, sl], pr[:vw, sl],
                            mu[h][:vw], rinv[h][:vw], sub_op, mul_op,
                        )
                        nc.vector.tensor_mul(of[:vw, sl], of[:vw, sl], wt[:vw])
                        nc.vector.tensor_add(of[:vw, sl], of[:vw, sl], bt[:vw])
                nc.gpsimd.dma_start(out=out_d[v0 : v0 + vw, :], in_=of[:vw])

    nc.compile()
    return nc


_NC_CACHE = {}


def _get_nc(trivial_affine):
    if trivial_affine not in _NC_CACHE:
        _NC_CACHE[trivial_affine] = _build_nc(trivial_affine)
    return _NC_CACHE[trivial_affine]


def _pack_pairs(arr, width):
    """[rows<=NP, cols<=width] -> [NKP, 128, 2, width] fp8 (zero padded)."""
    z = np.zeros((NP, width), dtype=np.float32)
    z[: arr.shape[0], : arr.shape[1]] = arr
    z = z.reshape(NKP, 2, 128, width).transpose(0, 2, 1, 3)
    return np.ascontiguousarray(np.clip(z, -240.0, 240.0)).astype(FP8)


def _prep_inputs(x, adp, W1, b1, W2, b2, ln_w, ln_b, idx):
    x = np.asarray(x, dtype=np.float32)
    adp = np.asarray(adp, dtype=np.float32)
    eye = np.eye(N, dtype=np.float32)

    def rownorm(a):
        a = a + eye
        return a / a.sum(axis=1, keepdims=True)

    A1 = rownorm(adp)
    A2 = rownorm(adp.T)
    a1_pk = _pack_pairs(A1.T * SCALE, NP)
    a2_pk = _pack_pairs(A2.T * SCALE, NP)

    W1 = np.asarray(W1, dtype=np.float32)
    W2 = np.asarray(W2, dtype=np.float32)
    beta = 1.0 - ALPHA
    W10, W11, W12 = W1[:, :C], W1[:, C : 2 * C], W1[:, 2 * C :]
    W20, W21, W22 = W2[:, :C], W2[:, C : 2 * C], W2[:, 2 * C :]
    U0 = W10 + ALPHA * (W11 + W12)
    U1 = beta * (W11 + ALPHA * W12)
    U2 = (beta ** 2) * W12
    V0 = W20 + ALPHA * (W21 + W22)
    V1 = beta * (W21 + ALPHA * W22)
    V2 = (beta ** 2) * W22
    M0 = U0 + V0 + np.eye(C, dtype=np.float32)
    bias = np.asarray(b1, dtype=np.float32) + np.asarray(b2, dtype=np.float32)

    xc = x.reshape(B, C, N * T)

    def cmix(M):
        return np.matmul(M, xc)  # [B, C, N*T]

    def to_nf(a):  # [B,C,N*T] -> [B, N, C*T]
        return np.ascontiguousarray(
            a.reshape(B, C, N, T).transpose(0, 2, 1, 3).reshape(B, N, FT)
        )

    p = to_nf(cmix(U2))
    q = to_nf(cmix(U1))
    r = to_nf(cmix(V2))
    s = to_nf(cmix(V1))
    m0x = to_nf(cmix(M0) + bias[None, :, None])

    idx = np.asarray(idx)
    lw = np.asarray(ln_w, dtype=np.float32)[:, idx, :]
    lb = np.asarray(ln_b, dtype=np.float32)[:, idx, :]
    trivial = bool(np.all(lw == 1.0)) and bool(np.all(lb == 0.0))
    if not trivial:
        lnw = np.ascontiguousarray(lw.transpose(1, 0, 2).reshape(N, FT))
        lnb = np.ascontiguousarray(lb.transpose(1, 0, 2).reshape(N, FT))

    in_maps = []
    for c in range(8):
        b0, b1i = 2 * c, 2 * c + 1
        im = dict(
            a1=a1_pk,
            a2=a2_pk,
            p=_pack_pairs(np.hstack([p[b0], p[b1i]]), F2),
            r=_pack_pairs(np.hstack([r[b0], r[b1i]]), F2),
            q=np.hstack([q[b0], q[b1i]]).astype(FP8),
            s=np.hstack([s[b0], s[b1i]]).astype(FP8),
            m0x=np.hstack([m0x[b0], m0x[b1i]]).astype(BF16),
        )
        if not trivial:
            im["lnw"] = lnw
            im["lnb"] = lnb
        in_maps.append(im)
    return in_maps, trivial


def _run(inputs, trace=False):
    in_maps, trivial = _prep_inputs(**inputs)
    nc = _get_nc(trivial)
    res = run_bass_kernel_spmd(nc, in_maps, list(range(8)), trace=trace)
    outs = np.empty((B, C, N, T), dtype=np.float32)
    for c in range(8):
        o = np.asarray(res.results[c]["out"]).astype(np.float32)
        for h in range(2):
            outs[2 * c + h] = (
                o[:, FT * h : FT * (h + 1)].reshape(N, C, T).transpose(1, 0, 2)
            )
    return outs, res


def kernel(**inputs):
    out, _ = _run(inputs, trace=False)
    return out


# revision 8
# speedup vs baseline: 2.0850x; 1.0209x over previous
"""Trainium2 kernel for nn_DecoderLayer_16097537426031 (gnn_message_passing).

Math (reference):
  A1 = rownorm(adp + I), A2 = rownorm(adp.T + I)
  mixprop(x, A, W, b) = W0 h0 + W1 h1 + W2 h2 + b,  h0 = x,
    h1 = a x + B A x, h2 = a x + B A h1   (a=0.05, B=0.95)
  out_pre = mixprop(x,A1,W1,b1) + mixprop(x,A2,W2,b2) + x
  out = LayerNorm_{C,N,T}(out_pre) * ln_w[:, idx, :] + ln_b[:, idx, :]

Channel mixing (64x64) commutes with node contraction (A @ .), so with
  U0 = W10 + a(W11+W12), U1 = B(W11 + a W12), U2 = B^2 W12   (same V for W2)
  M0 = U0 + V0 + I
  out_pre = M0 x + A1 (U1 x + A1 (U2 x)) + A2 (V1 x + A2 (V2 x)) + b1 + b2
The small channel matmuls (2.6% of FLOPs) are folded on host; the four big
[3000x3000] @ [3000x1536] node-propagation matmuls per core (2 samples packed
along the free dim), the adds, and the LayerNorm run on device.
Data-parallel over batch: core c gets samples (2c, 2c+1).

Device-side layout: all four propagation matmuls run in fp8 (e4m3) with
MatmulPerfMode.DoubleRow (two 128-row k-subtiles per instruction). A is scaled
by S=4096 on host so its ~3e-4 entries land in fp8 normal range; psum results
are scaled back by 1/S in the fused (psum*1/S + other) consume op. A1/A2 stay
resident in SBUF (one direction at a time); z intermediates stay in SBUF as
fp8 pair-tiles. acc and pre round-trip DRAM in bf16.
"""

import numpy as np
import ml_dtypes

import concourse.bass as bass
import concourse.bacc as bacc
import concourse.mybir as mybir
from concourse.tile import TileContext
from concourse.bass_utils import run_bass_kernel_spmd

B, C, N, T = 16, 64, 3000, 12
GDEP = 2
ALPHA = 0.05
EPS = 1e-5
FT = C * T            # 768 per-sample free width
F2 = 2 * FT           # 1536: two samples packed
NE = float(C * N * T) # LayerNorm element count per sample
NP = 3072             # padded node/contraction dim (12 pairs of 256)
NKP = 12              # k-pair count (DoubleRow: 256 contraction rows each)
SCALE = 4096.0        # fp8 scaling of A
INV_S = 1.0 / SCALE
BF16 = ml_dtypes.bfloat16
FP8 = ml_dtypes.float8_e4m3  # TRN fp8e4 (max +-240)

VS = [(v * 128, min(128, N - v * 128)) for v in range((N + 127) // 128)]  # 24 v-subtiles


def _build_nc(trivial_affine: bool):
    nc = bacc.Bacc(num_swdge_queues=4)
    dt = mybir.dt
    f32, bf16, f8 = dt.float32, dt.bfloat16, dt.float8e4
    DR = mybir.MatmulPerfMode.DoubleRow
    sub_op, mul_op, add_op = (
        mybir.AluOpType.subtract,
        mybir.AluOpType.mult,
        mybir.AluOpType.add,
    )

    a1_in = nc.dram_tensor("a1", [NKP, 128, 2, NP], f8, kind="ExternalInput")
    a2_in = nc.dram_tensor("a2", [NKP, 128, 2, NP], f8, kind="ExternalInput")
    p_in = nc.dram_tensor("p", [NKP, 128, 2, F2], f8, kind="ExternalInput")
    r_in = nc.dram_tensor("r", [NKP, 128, 2, F2], f8, kind="ExternalInput")
    q_in = nc.dram_tensor("q", [N, F2], f8, kind="ExternalInput")
    s_in = nc.dram_tensor("s", [N, F2], f8, kind="ExternalInput")
    m0x_in = nc.dram_tensor("m0x", [N, F2], bf16, kind="ExternalInput")
    if not trivial_affine:
        lnw_in = nc.dram_tensor("lnw", [N, FT], f32, kind="ExternalInput")
        lnb_in = nc.dram_tensor("lnb", [N, FT], f32, kind="ExternalInput")
    out_d = nc.dram_tensor("out", [N, F2], bf16, kind="ExternalOutput")

    from contextlib import ExitStack

    with TileContext(nc) as tc:
        with ExitStack() as stack:
            ep = lambda **kw: stack.enter_context(tc.tile_pool(**kw))
            a_pool = ep(name="apool", bufs=14)
            x_pool = ep(name="xpool", bufs=12)
            z_pool = ep(name="zpool", bufs=13)
            qs_pool = ep(name="qs", bufs=2)
            mx_pool = ep(name="mx", bufs=2)
            accw_pool = ep(name="accw", bufs=2)
            accr_pool = ep(name="accr", bufs=2)
            prew_pool = ep(name="prew", bufs=2)
            prer_pool = ep(name="prer", bufs=3)
            out_pool = ep(name="outp", bufs=2)
            ln_pool = ep(name="lnp", bufs=4)
            red_pool = ep(name="red", bufs=4)
            scrap_pool = ep(name="scrap", bufs=2)
            misc_pool = ep(name="misc", bufs=2)
            ps_pool = ep(name="psmm", bufs=6, space="PSUM")
            ps_stat_pool = ep(name="psst", bufs=1, space="PSUM")
            ps_bc_pool = ep(name="psbc", bufs=1, space="PSUM")
            dacc_pool = ep(name="dacc", bufs=24, space="DRAM")
            dpre_pool = ep(name="dpre", bufs=24, space="DRAM")
            ones_col = misc_pool.tile([128, 1], f32, tag="ones_col")
            nc.vector.memset(ones_col, 1.0)
            ones_row = misc_pool.tile([1, 128], f32, tag="ones_row")
            nc.vector.memset(ones_row, 1.0)

            def load_pairs(pool, src, width, tag):
                tiles = []
                for j in range(NKP):
                    t = pool.tile([128, 2, width], f8, tag=tag, name=f"{tag}_{j}")
                    nc.gpsimd.dma_start(out=t, in_=src[j])
                    tiles.append(t)
                return tiles

            def mm_phase(a_tiles, rhs_tiles, consume):
                """ps[f] = sum_j a_tiles[j].T-pairs @ rhs_tiles[j] f-chunks."""
                for vi, (v0, vw) in enumerate(VS):
                    ps = [
                        ps_pool.tile([128, 512], f32, tag="ps", name=f"ps_{vi}_{f}")
                        for f in range(3)
                    ]
                    for j in range(NKP):
                        lhsT = a_tiles[j][:, :, v0 : v0 + vw]
                        for f in range(3):
                            nc.tensor.matmul(
                                out=ps[f][:vw],
                                lhsT=lhsT,
                                rhs=rhs_tiles[j][:, :, 512 * f : 512 * (f + 1)],
                                start=(j == 0),
                                stop=(j == NKP - 1),
                                perf_mode=DR,
                            )
                    consume(vi, v0, vw, ps)

            # ---- Phase 1: z1 = Q + A1 @ P  (SBUF fp8 pair tiles) ----
            a1_t, p_t = [], []
            for j in range(NKP):
                t = a_pool.tile([128, 2, NP], f8, tag="a1", name=f"a1_{j}")
                nc.gpsimd.dma_start(out=t, in_=a1_in[j])
                a1_t.append(t)
                t = x_pool.tile([128, 2, F2], f8, tag="xp", name=f"xp_{j}")
                nc.gpsimd.dma_start(out=t, in_=p_in[j])
                p_t.append(t)
            z1_t = [
                z_pool.tile([128, 2, F2], f8, tag="z", name=f"z1_{j}")
                for j in range(NKP)
            ]
            # rows 3000..3071 of the k dim must be zero when z1 is used as rhs
            # (partition base must be 32-aligned, so clear the whole last tile
            # before the consumes overwrite the valid rows)
            nc.vector.memset(z1_t[NKP - 1], 0.0)

            def ph1(vi, v0, vw, ps):
                qt = qs_pool.tile([128, F2], f8, tag="qs")
                nc.sync.dma_start(out=qt[:vw], in_=q_in[v0 : v0 + vw, :])
                j, h = divmod(vi, 2)
                for f in range(3):
                    sl = slice(512 * f, 512 * (f + 1))
                    nc.vector.scalar_tensor_tensor(
                        out=z1_t[j][:vw, h, sl],
                        in0=ps[f][:vw],
                        scalar=INV_S,
                        in1=qt[:vw, sl],
                        op0=mul_op,
                        op1=add_op,
                    )

            mm_phase(a1_t, p_t, ph1)

            # ---- Phase 2: acc = M0X + A1 @ z1 -> DRAM bf16 ----
            acc_dram = [None] * len(VS)

            def ph2(vi, v0, vw, ps):
                mt = mx_pool.tile([128, F2], bf16, tag="mx")
                nc.sync.dma_start(out=mt[:vw], in_=m0x_in[v0 : v0 + vw, :])
                at = accw_pool.tile([128, F2], bf16, tag="accw")
                for f in range(3):
                    sl = slice(512 * f, 512 * (f + 1))
                    nc.vector.scalar_tensor_tensor(
                        out=at[:vw, sl],
                        in0=ps[f][:vw],
                        scalar=INV_S,
                        in1=mt[:vw, sl],
                        op0=mul_op,
                        op1=add_op,
                    )
                d = dacc_pool.tile([128, F2], bf16, tag="dacc")
                nc.gpsimd.dma_start(out=d[:vw], in_=at[:vw])
                acc_dram[vi] = d

            mm_phase(a1_t, z1_t, ph2)

            # ---- Phase 3: z3 = S + A2 @ R ----
            a2_t = load_pairs(a_pool, a2_in, NP, "a1")
            r_t = load_pairs(x_pool, r_in, F2, "xp")
            z3_t = [
                z_pool.tile([128, 2, F2], f8, tag="z", name=f"z3_{j}")
                for j in range(NKP)
            ]
            nc.vector.memset(z3_t[NKP - 1], 0.0)

            def ph3(vi, v0, vw, ps):
                st = qs_pool.tile([128, F2], f8, tag="qs")
                nc.sync.dma_start(out=st[:vw], in_=s_in[v0 : v0 + vw, :])
                j, h = divmod(vi, 2)
                for f in range(3):
                    sl = slice(512 * f, 512 * (f + 1))
                    nc.vector.scalar_tensor_tensor(
                        out=z3_t[j][:vw, h, sl],
                        in0=ps[f][:vw],
                        scalar=INV_S,
                        in1=st[:vw, sl],
                        op0=mul_op,
                        op1=add_op,
                    )

            mm_phase(a2_t, r_t, ph3)

            # ---- Phase 4: pre = acc + A2 @ z3 -> DRAM bf16, + LN stats ----
            pre_dram = [None] * len(VS)
            pre_sbuf = [None] * len(VS)
            stat_ps = ps_stat_pool.tile([1, 6], f32, tag="psst")
            # stt chunks: (ps idx, ps col slice, out col slice, red col)
            CH = [
                (0, slice(0, 512), slice(0, 512), 0),
                (1, slice(0, 256), slice(512, 768), 1),
                (1, slice(256, 512), slice(768, 1024), 2),
                (2, slice(0, 512), slice(1024, 1536), 3),
            ]

            def ph4(vi, v0, vw, ps):
                ar = accr_pool.tile([128, F2], bf16, tag="accr")
                nc.sync.dma_start(out=ar[:vw], in_=acc_dram[vi][:vw])
                if vi < 8:
                    pt = prew_pool.tile([128, F2], bf16, tag="prew")
                elif vi < 10:
                    pt = mx_pool.tile([128, F2], bf16, tag="mx", name=f"pre_sb_{vi}")
                elif vi < 12:
                    pt = accw_pool.tile([128, F2], bf16, tag="accw", name=f"pre_sb_{vi}")
                else:
                    pt = x_pool.tile([128, F2], bf16, tag="xp", name=f"pre_sb_{vi}")
                red = red_pool.tile([128, 6], f32, tag="red")
                for pi, psl, osl, rc in CH:
                    nc.vector.scalar_tensor_tensor(
                        out=pt[:vw, osl],
                        in0=ps[pi][:vw, psl],
                        scalar=INV_S,
                        in1=ar[:vw, osl],
                        op0=mul_op,
                        op1=add_op,
                        accum_out=red[:vw, rc : rc + 1],
                    )
                sc0 = scrap_pool.tile([128, FT], bf16, tag="scrap")
                nc.scalar.activation(
                    sc0[:vw], pt[:vw, 0:FT],
                    mybir.ActivationFunctionType.Square,
                    accum_out=red[:vw, 4:5],
                )
                sc1 = scrap_pool.tile([128, FT], bf16, tag="scrap")
                nc.scalar.activation(
                    sc1[:vw], pt[:vw, FT:F2],
                    mybir.ActivationFunctionType.Square,
                    accum_out=red[:vw, 5:6],
                )
                nc.tensor.matmul(
                    out=stat_ps[0:1, 0:6],
                    lhsT=ones_col[:vw, 0:1],
                    rhs=red[:vw, 0:6],
                    start=(vi == 0),
                    stop=(vi == len(VS) - 1),
                )
                if vi < 8:
                    d = dpre_pool.tile([128, F2], bf16, tag="dpre")
                    nc.gpsimd.dma_start(out=d[:vw], in_=pt[:vw])
                    pre_dram[vi] = d
                else:
                    pre_sbuf[vi] = pt

            mm_phase(a2_t, z3_t, ph4)

            # ---- Finalize stats: mu, rinv per sample; broadcast to 128 ----
            stat_sb = misc_pool.tile([1, 6], f32, tag="stat_sb")
            nc.vector.tensor_copy(stat_sb, stat_ps[0:1, 0:6])
            sums2 = misc_pool.tile([1, 2], f32, tag="sums2")
            nc.vector.tensor_add(sums2[:, 0:1], stat_sb[:, 0:1], stat_sb[:, 1:2])
            nc.vector.tensor_add(sums2[:, 1:2], stat_sb[:, 2:3], stat_sb[:, 3:4])
            mean2 = misc_pool.tile([1, 2], f32, tag="mean2")
            nc.scalar.mul(mean2, sums2, 1.0 / NE)
            ex2 = misc_pool.tile([1, 2], f32, tag="ex2")
            nc.scalar.mul(ex2, stat_sb[:, 4:6], 1.0 / NE)
            musq = misc_pool.tile([1, 2], f32, tag="musq")
            nc.scalar.square(musq, mean2)
            veps = misc_pool.tile([1, 2], f32, tag="veps")
            nc.vector.tensor_sub(veps, ex2, musq)
            nc.vector.tensor_scalar_add(veps, veps, EPS)
            rec = misc_pool.tile([1, 2], f32, tag="rec")
            nc.vector.reciprocal(rec, veps)
            fin = misc_pool.tile([1, 4], f32, tag="fin")
            nc.scalar.copy(fin[:, 0:2], mean2)
            nc.scalar.sqrt(fin[:, 2:4], rec)
            bc_ps = ps_bc_pool.tile([128, 4], f32, tag="psbc")
            nc.tensor.matmul(
                out=bc_ps, lhsT=ones_row[0:1, 0:128], rhs=fin[0:1, 0:4],
                start=True, stop=True,
            )
            bc = misc_pool.tile([128, 4], f32, tag="bc")
            nc.vector.tensor_copy(bc, bc_ps)
            mu = [bc[:, 0:1], bc[:, 1:2]]
            rinv = [bc[:, 2:3], bc[:, 3:4]]

            # ---- Phase 5: normalize (+ affine) -> out ----
            for vi in list(range(8, len(VS))) + list(range(8)):
                v0, vw = VS[vi]
                if pre_sbuf[vi] is not None:
                    pr = pre_sbuf[vi]
                else:
                    pr = prer_pool.tile([128, F2], bf16, tag="prer")
                    nc.sync.dma_start(out=pr[:vw], in_=pre_dram[vi][:vw])
                of = out_pool.tile([128, F2], bf16, tag="outp")
                if trivial_affine:
                    for h in range(2):
                        sl = slice(FT * h, FT * (h + 1))
                        nc.vector.tensor_scalar(
                            of[:vw, sl], pr[:vw, sl],
                            mu[h][:vw], rinv[h][:vw], sub_op, mul_op,
                        )
                else:
                    wt = ln_pool.tile([128, FT], f32, tag="lnw")
                    nc.gpsimd.dma_start(out=wt[:vw], in_=lnw_in[v0 : v0 + vw, :])
                    bt = ln_pool.tile([128, FT], f32, tag="lnb")
                    nc.gpsimd.dma_start(out=bt[:vw], in_=lnb_in[v0 : v0 + vw, :])
                    for h in range(2):
                        sl = slice(FT * h, FT * (h + 1))
                        nc.vector.tensor_scalar(
                            of[:vw, sl], pr[:vw, sl],
                            mu[h][:vw], rinv[h][:vw], sub_op, mul_op,
                        )
                        nc.vector.tensor_mul(of[:vw, sl], of[:vw, sl], wt[:vw])
                        nc.vector.tensor_add(of[:vw, sl], of[:vw, sl], bt[:vw])
                nc.sync.dma_start(out=out_d[v0 : v0 + vw, :], in_=of[:vw])

    nc.compile()
    return nc


_NC_CACHE = {}


def _get_nc(trivial_affine):
    if trivial_affine not in _NC_CACHE:
        _NC_CACHE[trivial_affine] = _build_nc(trivial_affine)
    return _NC_CACHE[trivial_affine]


def _pack_pairs(arr, width):
    """[rows<=NP, cols<=width] -> [NKP, 128, 2, width] fp8 (zero padded)."""
    z = np.zeros((NP, width), dtype=np.float32)
    z[: arr.shape[0], : arr.shape[1]] = arr
    z = z.reshape(NKP, 2, 128, width).transpose(0, 2, 1, 3)
    return np.ascontiguousarray(np.clip(z, -240.0, 240.0)).astype(FP8)


def _prep_inputs(x, adp, W1, b1, W2, b2, ln_w, ln_b, idx):
    x = np.asarray(x, dtype=np.float32)
    adp = np.asarray(adp, dtype=np.float32)
    eye = np.eye(N, dtype=np.float32)

    def rownorm(a):
        a = a + eye
        return a / a.sum(axis=1, keepdims=True)

    A1 = rownorm(adp)
    A2 = rownorm(adp.T)
    a1_pk = _pack_pairs(A1.T * SCALE, NP)
    a2_pk = _pack_pairs(A2.T * SCALE, NP)

    W1 = np.asarray(W1, dtype=np.float32)
    W2 = np.asarray(W2, dtype=np.float32)
    beta = 1.0 - ALPHA
    W10, W11, W12 = W1[:, :C], W1[:, C : 2 * C], W1[:, 2 * C :]
    W20, W21, W22 = W2[:, :C], W2[:, C : 2 * C], W2[:, 2 * C :]
    U0 = W10 + ALPHA * (W11 + W12)
    U1 = beta * (W11 + ALPHA * W12)
    U2 = (beta ** 2) * W12
    V0 = W20 + ALPHA * (W21 + W22)
    V1 = beta * (W21 + ALPHA * W22)
    V2 = (beta ** 2) * W22
    M0 = U0 + V0 + np.eye(C, dtype=np.float32)
    bias = np.asarray(b1, dtype=np.float32) + np.asarray(b2, dtype=np.float32)

    xc = x.reshape(B, C, N * T)

    def cmix(M):
        return np.matmul(M, xc)  # [B, C, N*T]

    def to_nf(a):  # [B,C,N*T] -> [B, N, C*T]
        return np.ascontiguousarray(
            a.reshape(B, C, N, T).transpose(0, 2, 1, 3).reshape(B, N, FT)
        )

    p = to_nf(cmix(U2))
    q = to_nf(cmix(U1))
    r = to_nf(cmix(V2))
    s = to_nf(cmix(V1))
    m0x = to_nf(cmix(M0) + bias[None, :, None])

    idx = np.asarray(idx)
    lw = np.asarray(ln_w, dtype=np.float32)[:, idx, :]
    lb = np.asarray(ln_b, dtype=np.float32)[:, idx, :]
    trivial = bool(np.all(lw == 1.0)) and bool(np.all(lb == 0.0))
    if not trivial:
        lnw = np.ascontiguousarray(lw.transpose(1, 0, 2).reshape(N, FT))
        lnb = np.ascontiguousarray(lb.transpose(1, 0, 2).reshape(N, FT))

    in_maps = []
    for c in range(8):
        b0, b1i = 2 * c, 2 * c + 1
        im = dict(
            a1=a1_pk,
            a2=a2_pk,
            p=_pack_pairs(np.hstack([p[b0], p[b1i]]), F2),
            r=_pack_pairs(np.hstack([r[b0], r[b1i]]), F2),
            q=np.hstack([q[b0], q[b1i]]).astype(FP8),
            s=np.hstack([s[b0], s[b1i]]).astype(FP8),
            m0x=np.hstack([m0x[b0], m0x[b1i]]).astype(BF16),
        )
        if not trivial:
            im["lnw"] = lnw
            im["lnb"] = lnb
        in_maps.append(im)
    return in_maps, trivial


def _run(inputs, trace=False):
    in_maps, trivial = _prep_inputs(**inputs)
    nc = _get_nc(trivial)
    res = run_bass_kernel_spmd(nc, in_maps, list(range(8)), trace=trace)
    outs = np.empty((B, C, N, T), dtype=np.float32)
    for c in range(8):
        o = np.asarray(res.results[c]["out"]).astype(np.float32)
        for h in range(2):
            outs[2 * c + h] = (
                o[:, FT * h : FT * (h + 1)].reshape(N, C, T).transpose(1, 0, 2)
            )
    return outs, res


def kernel(**inputs):
    out, _ = _run(inputs, trace=False)
    return out


# revision 10
# speedup vs baseline: 2.1152x; 1.0145x over previous
"""Trainium2 kernel for nn_DecoderLayer_16097537426031 (gnn_message_passing).

Math (reference):
  A1 = rownorm(adp + I), A2 = rownorm(adp.T + I)
  mixprop(x, A, W, b) = W0 h0 + W1 h1 + W2 h2 + b,  h0 = x,
    h1 = a x + B A x, h2 = a x + B A h1   (a=0.05, B=0.95)
  out_pre = mixprop(x,A1,W1,b1) + mixprop(x,A2,W2,b2) + x
  out = LayerNorm_{C,N,T}(out_pre) * ln_w[:, idx, :] + ln_b[:, idx, :]

Channel mixing (64x64) commutes with node contraction (A @ .), so with
  U0 = W10 + a(W11+W12), U1 = B(W11 + a W12), U2 = B^2 W12   (same V for W2)
  M0 = U0 + V0 + I
  out_pre = M0 x + A1 (U1 x + A1 (U2 x)) + A2 (V1 x + A2 (V2 x)) + b1 + b2
The small channel matmuls (2.6% of FLOPs) are folded on host; the four big
[3000x3000] @ [3000x1536] node-propagation matmuls per core (2 samples packed
along the free dim), the adds, and the LayerNorm run on device.
Data-parallel over batch: core c gets samples (2c, 2c+1).

Device-side layout: all four propagation matmuls run in fp8 (e4m3) with
MatmulPerfMode.DoubleRow (two 128-row k-subtiles per instruction). A is scaled
by S=4096 on host so its ~3e-4 entries land in fp8 normal range; psum results
are scaled back by 1/S in the fused (psum*1/S + other) consume op. A1/A2 stay
resident in SBUF (one direction at a time); z intermediates stay in SBUF as
fp8 pair-tiles. acc and pre round-trip DRAM in bf16.
"""

import numpy as np
import ml_dtypes

import concourse.bass as bass
import concourse.bacc as bacc
import concourse.mybir as mybir
from concourse.tile import TileContext
from concourse.bass_utils import run_bass_kernel_spmd

B, C, N, T = 16, 64, 3000, 12
GDEP = 2
ALPHA = 0.05
EPS = 1e-5
FT = C * T            # 768 per-sample free width
F2 = 2 * FT           # 1536: two samples packed
NE = float(C * N * T) # LayerNorm element count per sample
NP = 3072             # padded node/contraction dim (12 pairs of 256)
NKP = 12              # k-pair count (DoubleRow: 256 contraction rows each)
SCALE = 4096.0        # fp8 scaling of A
INV_S = 1.0 / SCALE
BF16 = ml_dtypes.bfloat16
FP8 = ml_dtypes.float8_e4m3  # TRN fp8e4 (max +-240)

VS = [(v * 128, min(128, N - v * 128)) for v in range((N + 127) // 128)]  # 24 v-subtiles


def _build_nc(trivial_affine: bool):
    nc = bacc.Bacc(num_swdge_queues=4)
    dt = mybir.dt
    f32, bf16, f8 = dt.float32, dt.bfloat16, dt.float8e4
    DR = mybir.MatmulPerfMode.DoubleRow
    sub_op, mul_op, add_op = (
        mybir.AluOpType.subtract,
        mybir.AluOpType.mult,
        mybir.AluOpType.add,
    )

    a1_in = nc.dram_tensor("a1", [NKP, 128, 2, NP], f8, kind="ExternalInput")
    a2_in = nc.dram_tensor("a2", [NKP, 128, 2, NP], f8, kind="ExternalInput")
    p_in = nc.dram_tensor("p", [NKP, 128, 2, F2], f8, kind="ExternalInput")
    r_in = nc.dram_tensor("r", [NKP, 128, 2, F2], f8, kind="ExternalInput")
    q_in = nc.dram_tensor("q", [N, F2], f8, kind="ExternalInput")
    s_in = nc.dram_tensor("s", [N, F2], f8, kind="ExternalInput")
    m0x_in = nc.dram_tensor("m0x", [N, F2], bf16, kind="ExternalInput")
    if not trivial_affine:
        lnw_in = nc.dram_tensor("lnw", [N, FT], f32, kind="ExternalInput")
        lnb_in = nc.dram_tensor("lnb", [N, FT], f32, kind="ExternalInput")
    out_d = nc.dram_tensor("out", [NKP, 2, 128, F2], bf16, kind="ExternalOutput")

    from contextlib import ExitStack

    with TileContext(nc) as tc:
        with ExitStack() as stack:
            ep = lambda **kw: stack.enter_context(tc.tile_pool(**kw))
            a_pool = ep(name="apool", bufs=14)
            x_pool = ep(name="xpool", bufs=12)
            z_pool = ep(name="zpool", bufs=12)
            qs_pool = ep(name="qs", bufs=2)
            mx_pool = ep(name="mx", bufs=2)
            accw_pool = ep(name="accw", bufs=2)
            accr_pool = ep(name="accr", bufs=2)
            prew_pool = ep(name="prew", bufs=1)
            prer_pool = ep(name="prer", bufs=3)
            out_pool = ep(name="outp", bufs=2)
            ln_pool = ep(name="lnp", bufs=4)
            red_pool = ep(name="red", bufs=4)
            scrap_pool = ep(name="scrap", bufs=1)
            misc_pool = ep(name="misc", bufs=2)
            ps_pool = ep(name="psmm", bufs=6, space="PSUM")
            ps_stat_pool = ep(name="psst", bufs=1, space="PSUM")
            ps_bc_pool = ep(name="psbc", bufs=1, space="PSUM")
            dacc_pool = ep(name="dacc", bufs=24, space="DRAM")
            dpre_pool = ep(name="dpre", bufs=24, space="DRAM")
            ones_col = misc_pool.tile([128, 1], f32, tag="ones_col")
            nc.vector.memset(ones_col, 1.0)
            ones_row = misc_pool.tile([1, 128], f32, tag="ones_row")
            nc.vector.memset(ones_row, 1.0)

            def load_pairs(pool, src, width, tag):
                tiles = []
                for j in range(NKP):
                    t = pool.tile([128, 2, width], f8, tag=tag, name=f"{tag}_{j}")
                    nc.gpsimd.dma_start(out=t, in_=src[j])
                    tiles.append(t)
                return tiles

            def mm_phase(a_tiles, rhs_tiles, consume):
                """ps[f] = sum_j a_tiles[j].T-pairs @ rhs_tiles[j] f-chunks."""
                for vi, (v0, vw) in enumerate(VS):
                    ps = [
                        ps_pool.tile([128, 512], f32, tag="ps", name=f"ps_{vi}_{f}")
                        for f in range(3)
                    ]
                    for j in range(NKP):
                        lhsT = a_tiles[j][:, :, v0 : v0 + vw]
                        for f in range(3):
                            nc.tensor.matmul(
                                out=ps[f][:vw],
                                lhsT=lhsT,
                                rhs=rhs_tiles[j][:, :, 512 * f : 512 * (f + 1)],
                                start=(j == 0),
                                stop=(j == NKP - 1),
                                perf_mode=DR,
                            )
                    consume(vi, v0, vw, ps)

            # ---- Phase 1: z1 = Q + A1 @ P  (SBUF fp8 pair tiles) ----
            a1_t, p_t = [], []
            for j in range(NKP):
                t = a_pool.tile([128, 2, NP], f8, tag="a1", name=f"a1_{j}")
                nc.gpsimd.dma_start(out=t, in_=a1_in[j])
                a1_t.append(t)
                t = x_pool.tile([128, 2, F2], f8, tag="xp", name=f"xp_{j}")
                nc.gpsimd.dma_start(out=t, in_=p_in[j])
                p_t.append(t)
            z1_t = [
                z_pool.tile([128, 2, F2], f8, tag="z", name=f"z1_{j}")
                for j in range(NKP)
            ]
            # rows 3000..3071 of the k dim must be zero when z1 is used as rhs
            # (partition base must be 32-aligned, so clear the whole last tile
            # before the consumes overwrite the valid rows)
            nc.vector.memset(z1_t[NKP - 1], 0.0)

            def ph1(vi, v0, vw, ps):
                qt = qs_pool.tile([128, F2], f8, tag="qs")
                nc.sync.dma_start(out=qt[:vw], in_=q_in[v0 : v0 + vw, :])
                j, h = divmod(vi, 2)
                for f in range(3):
                    sl = slice(512 * f, 512 * (f + 1))
                    nc.vector.scalar_tensor_tensor(
                        out=z1_t[j][:vw, h, sl],
                        in0=ps[f][:vw],
                        scalar=INV_S,
                        in1=qt[:vw, sl],
                        op0=mul_op,
                        op1=add_op,
                    )

            mm_phase(a1_t, p_t, ph1)

            # ---- Phase 2: acc = M0X + A1 @ z1 -> DRAM bf16 ----
            acc_dram = [None] * len(VS)

            def ph2(vi, v0, vw, ps):
                mt = mx_pool.tile([128, F2], bf16, tag="mx")
                nc.sync.dma_start(out=mt[:vw], in_=m0x_in[v0 : v0 + vw, :])
                at = accw_pool.tile([128, F2], bf16, tag="accw")
                for f in range(3):
                    sl = slice(512 * f, 512 * (f + 1))
                    nc.vector.scalar_tensor_tensor(
                        out=at[:vw, sl],
                        in0=ps[f][:vw],
                        scalar=INV_S,
                        in1=mt[:vw, sl],
                        op0=mul_op,
                        op1=add_op,
                    )
                d = dacc_pool.tile([128, F2], bf16, tag="dacc")
                nc.gpsimd.dma_start(out=d[:vw], in_=at[:vw])
                acc_dram[vi] = d

            mm_phase(a1_t, z1_t, ph2)

            # ---- Phase 3: z3 = S + A2 @ R ----
            a2_t = load_pairs(a_pool, a2_in, NP, "a1")
            r_t = load_pairs(x_pool, r_in, F2, "xp")
            z3_t = [
                z_pool.tile([128, 2, F2], f8, tag="z", name=f"z3_{j}")
                for j in range(NKP)
            ]
            nc.vector.memset(z3_t[NKP - 1], 0.0)

            def ph3(vi, v0, vw, ps):
                st = qs_pool.tile([128, F2], f8, tag="qs")
                nc.sync.dma_start(out=st[:vw], in_=s_in[v0 : v0 + vw, :])
                j, h = divmod(vi, 2)
                for f in range(3):
                    sl = slice(512 * f, 512 * (f + 1))
                    nc.vector.scalar_tensor_tensor(
                        out=z3_t[j][:vw, h, sl],
                        in0=ps[f][:vw],
                        scalar=INV_S,
                        in1=st[:vw, sl],
                        op0=mul_op,
                        op1=add_op,
                    )

            mm_phase(a2_t, r_t, ph3)

            # ---- Phase 4: pre = acc + A2 @ z3 -> DRAM bf16, + LN stats ----
            pre_dram = [None] * len(VS)
            pre_sbuf = [None] * len(VS)
            stat_ps = ps_stat_pool.tile([1, 6], f32, tag="psst")
            # stt chunks: (ps idx, ps col slice, out col slice, red col)
            CH = [
                (0, slice(0, 512), slice(0, 512), 0),
                (1, slice(0, 256), slice(512, 768), 1),
                (1, slice(256, 512), slice(768, 1024), 2),
                (2, slice(0, 512), slice(1024, 1536), 3),
            ]

            def ph4(vi, v0, vw, ps):
                ar = accr_pool.tile([128, F2], bf16, tag="accr")
                nc.sync.dma_start(out=ar[:vw], in_=acc_dram[vi][:vw])
                if vi < 8:
                    pt = prew_pool.tile([128, F2], bf16, tag="prew")
                elif vi < 10:
                    pt = mx_pool.tile([128, F2], bf16, tag="mx", name=f"pre_sb_{vi}")
                elif vi < 12:
                    pt = accw_pool.tile([128, F2], bf16, tag="accw", name=f"pre_sb_{vi}")
                else:
                    pt = x_pool.tile([128, F2], bf16, tag="xp", name=f"pre_sb_{vi}")
                red = red_pool.tile([128, 6], f32, tag="red")
                for pi, psl, osl, rc in CH:
                    nc.vector.scalar_tensor_tensor(
                        out=pt[:vw, osl],
                        in0=ps[pi][:vw, psl],
                        scalar=INV_S,
                        in1=ar[:vw, osl],
                        op0=mul_op,
                        op1=add_op,
                        accum_out=red[:vw, rc : rc + 1],
                    )
                sc0 = scrap_pool.tile([128, FT], bf16, tag="scrap")
                nc.scalar.activation(
                    sc0[:vw], pt[:vw, 0:FT],
                    mybir.ActivationFunctionType.Square,
                    accum_out=red[:vw, 4:5],
                )
                sc1 = scrap_pool.tile([128, FT], bf16, tag="scrap")
                nc.scalar.activation(
                    sc1[:vw], pt[:vw, FT:F2],
                    mybir.ActivationFunctionType.Square,
                    accum_out=red[:vw, 5:6],
                )
                nc.tensor.matmul(
                    out=stat_ps[0:1, 0:6],
                    lhsT=ones_col[:vw, 0:1],
                    rhs=red[:vw, 0:6],
                    start=(vi == 0),
                    stop=(vi == len(VS) - 1),
                )
                if vi < 8:
                    d = dpre_pool.tile([128, F2], bf16, tag="dpre")
                    nc.gpsimd.dma_start(out=d[:vw], in_=pt[:vw])
                    pre_dram[vi] = d
                else:
                    pre_sbuf[vi] = pt

            mm_phase(a2_t, z3_t, ph4)

            # ---- Finalize stats: mu, rinv per sample; broadcast to 128 ----
            stat_sb = misc_pool.tile([1, 6], f32, tag="stat_sb")
            nc.vector.tensor_copy(stat_sb, stat_ps[0:1, 0:6])
            sums2 = misc_pool.tile([1, 2], f32, tag="sums2")
            nc.vector.tensor_add(sums2[:, 0:1], stat_sb[:, 0:1], stat_sb[:, 1:2])
            nc.vector.tensor_add(sums2[:, 1:2], stat_sb[:, 2:3], stat_sb[:, 3:4])
            mean2 = misc_pool.tile([1, 2], f32, tag="mean2")
            nc.scalar.mul(mean2, sums2, 1.0 / NE)
            ex2 = misc_pool.tile([1, 2], f32, tag="ex2")
            nc.scalar.mul(ex2, stat_sb[:, 4:6], 1.0 / NE)
            musq = misc_pool.tile([1, 2], f32, tag="musq")
            nc.scalar.square(musq, mean2)
            veps = misc_pool.tile([1, 2], f32, tag="veps")
            nc.vector.tensor_sub(veps, ex2, musq)
            nc.vector.tensor_scalar_add(veps, veps, EPS)
            rec = misc_pool.tile([1, 2], f32, tag="rec")
            nc.vector.reciprocal(rec, veps)
            fin = misc_pool.tile([1, 4], f32, tag="fin")
            nc.scalar.copy(fin[:, 0:2], mean2)
            nc.scalar.sqrt(fin[:, 2:4], rec)
            bc_ps = ps_bc_pool.tile([128, 4], f32, tag="psbc")
            nc.tensor.matmul(
                out=bc_ps, lhsT=ones_row[0:1, 0:128], rhs=fin[0:1, 0:4],
                start=True, stop=True,
            )
            bc = misc_pool.tile([128, 4], f32, tag="bc")
            nc.vector.tensor_copy(bc, bc_ps)
            mu = [bc[:, 0:1], bc[:, 1:2]]
            rinv = [bc[:, 2:3], bc[:, 3:4]]

            # ---- Phase 5: normalize (+ affine) -> out (paired stores) ----
            for g in list(range(4, NKP)) + list(range(4)):
                of = out_pool.tile([128, 2, F2], bf16, tag="outp", name=f"of_{g}")
                for hh in range(2):
                    vi = 2 * g + hh
                    v0, vw = VS[vi]
                    if pre_sbuf[vi] is not None:
                        pr = pre_sbuf[vi]
                    else:
                        pr = prer_pool.tile([128, F2], bf16, tag="prer")
                        nc.sync.dma_start(out=pr[:vw], in_=pre_dram[vi][:vw])
                    for h in range(2):
                        sl = slice(FT * h, FT * (h + 1))
                        nc.vector.tensor_scalar(
                            of[:vw, hh, sl], pr[:vw, sl],
                            mu[h][:vw], rinv[h][:vw], sub_op, mul_op,
                        )
                        if not trivial_affine:
                            wt = ln_pool.tile([128, FT], f32, tag="lnw")
                            nc.sync.dma_start(out=wt[:vw], in_=lnw_in[v0 : v0 + vw, :])
                            bt = ln_pool.tile([128, FT], f32, tag="lnb")
                            nc.sync.dma_start(out=bt[:vw], in_=lnb_in[v0 : v0 + vw, :])
                            nc.vector.tensor_mul(of[:vw, hh, sl], of[:vw, hh, sl], wt[:vw])
                            nc.vector.tensor_add(of[:vw, hh, sl], of[:vw, hh, sl], bt[:vw])
                nc.sync.dma_start(
                    out=out_d[g].rearrange("h p f -> p h f"), in_=of
                )

    nc.compile()
    return nc


_NC_CACHE = {}


def _get_nc(trivial_affine):
    if trivial_affine not in _NC_CACHE:
        _NC_CACHE[trivial_affine] = _build_nc(trivial_affine)
    return _NC_CACHE[trivial_affine]


def _pack_pairs(arr, width):
    """[rows<=NP, cols<=width] -> [NKP, 128, 2, width] fp8 (zero padded)."""
    z = np.zeros((NP, width), dtype=np.float32)
    z[: arr.shape[0], : arr.shape[1]] = arr
    z = z.reshape(NKP, 2, 128, width).transpose(0, 2, 1, 3)
    return np.ascontiguousarray(np.clip(z, -240.0, 240.0)).astype(FP8)


def _prep_inputs(x, adp, W1, b1, W2, b2, ln_w, ln_b, idx):
    x = np.asarray(x, dtype=np.float32)
    adp = np.asarray(adp, dtype=np.float32)
    eye = np.eye(N, dtype=np.float32)

    def rownorm(a):
        a = a + eye
        return a / a.sum(axis=1, keepdims=True)

    A1 = rownorm(adp)
    A2 = rownorm(adp.T)
    a1_pk = _pack_pairs(A1.T * SCALE, NP)
    a2_pk = _pack_pairs(A2.T * SCALE, NP)

    W1 = np.asarray(W1, dtype=np.float32)
    W2 = np.asarray(W2, dtype=np.float32)
    beta = 1.0 - ALPHA
    W10, W11, W12 = W1[:, :C], W1[:, C : 2 * C], W1[:, 2 * C :]
    W20, W21, W22 = W2[:, :C], W2[:, C : 2 * C], W2[:, 2 * C :]
    U0 = W10 + ALPHA * (W11 + W12)
    U1 = beta * (W11 + ALPHA * W12)
    U2 = (beta ** 2) * W12
    V0 = W20 + ALPHA * (W21 + W22)
    V1 = beta * (W21 + ALPHA * W22)
    V2 = (beta ** 2) * W22
    M0 = U0 + V0 + np.eye(C, dtype=np.float32)
    bias = np.asarray(b1, dtype=np.float32) + np.asarray(b2, dtype=np.float32)

    xc = x.reshape(B, C, N * T)

    def cmix(M):
        return np.matmul(M, xc)  # [B, C, N*T]

    def to_nf(a):  # [B,C,N*T] -> [B, N, C*T]
        return np.ascontiguousarray(
            a.reshape(B, C, N, T).transpose(0, 2, 1, 3).reshape(B, N, FT)
        )

    p = to_nf(cmix(U2))
    q = to_nf(cmix(U1))
    r = to_nf(cmix(V2))
    s = to_nf(cmix(V1))
    m0x = to_nf(cmix(M0) + bias[None, :, None])

    idx = np.asarray(idx)
    lw = np.asarray(ln_w, dtype=np.float32)[:, idx, :]
    lb = np.asarray(ln_b, dtype=np.float32)[:, idx, :]
    trivial = bool(np.all(lw == 1.0)) and bool(np.all(lb == 0.0))
    if not trivial:
        lnw = np.ascontiguousarray(lw.transpose(1, 0, 2).reshape(N, FT))
        lnb = np.ascontiguousarray(lb.transpose(1, 0, 2).reshape(N, FT))

    in_maps = []
    for c in range(8):
        b0, b1i = 2 * c, 2 * c + 1
        im = dict(
            a1=a1_pk,
            a2=a2_pk,
            p=_pack_pairs(np.hstack([p[b0], p[b1i]]), F2),
            r=_pack_pairs(np.hstack([r[b0], r[b1i]]), F2),
            q=np.hstack([q[b0], q[b1i]]).astype(FP8),
            s=np.hstack([s[b0], s[b1i]]).astype(FP8),
            m0x=np.hstack([m0x[b0], m0x[b1i]]).astype(BF16),
        )
        if not trivial:
            im["lnw"] = lnw
            im["lnb"] = lnb
        in_maps.append(im)
    return in_maps, trivial


def _run(inputs, trace=False):
    in_maps, trivial = _prep_inputs(**inputs)
    nc = _get_nc(trivial)
    res = run_bass_kernel_spmd(nc, in_maps, list(range(8)), trace=trace)
    outs = np.empty((B, C, N, T), dtype=np.float32)
    for c in range(8):
        o = (
            np.asarray(res.results[c]["out"])
            .astype(np.float32)
            .reshape(NP, F2)[:N]
        )
        for h in range(2):
            outs[2 * c + h] = (
                o[:, FT * h : FT * (h + 1)].reshape(N, C, T).transpose(1, 0, 2)
            )
    return outs, res


def kernel(**inputs):
    out, _ = _run(inputs, trace=False)
    return out
